# revision 1
# baseline (speedup 1.0000x reference)
"""GCN (2-layer, PyG-style gcn_norm) Bass/Tile kernel for Trainium2, 8 NeuronCores.

Strategy (dst-partitioned message passing):
  - Nodes are partitioned across 8 cores by destination; every edge is routed to
    the core that owns its destination node. Self-loops + symmetric D^-1/2 norm
    are computed on the host (index/routing preprocessing only).
  - Per core, edges are bucketed by source range (dma_gather indices are int16,
    so the feature table is addressed in <=32k-row buckets) and grouped by
    destination tile (128 dst nodes), packed into 128-message blocks.
  - Message features are bulk row-gathered from a replicated node-feature table
    in HBM with InstDMAGatherAnt (one call covers up to `gcols` blocks).
  - The segmented scatter-add becomes TensorE matmuls: for each block, a one-hot
    "selection" matrix S[e, d] = norm_e * (dst_local_e == d) is built with one
    DVE tensor_scalar op, and PSUM accumulates sum_e msg[e,:]^T S[e,:] over the
    tile's blocks. Bucket passes accumulate into a per-tile SBUF f32 buffer.
  - The layer weight matmul, bias+ReLU (ScalarE) and a TensorE transpose produce
    row-major output tiles, DMA'd to HBM.
  - Between layers, an AllGather shares the per-core H shards (the "halo
    exchange"); layer 2 gathers from the replicated table the same way.
"""

import os
from dataclasses import dataclass

import numpy as np

P = 128
NBUCK = 4  # source-range buckets (int16 gather indices => <=32768 rows each)


@dataclass(frozen=True)
class Geom:
    n_nodes: int
    n_cores: int
    in_dim: int
    h1: int
    h2: int
    gcols: int  # gather-group size, in 128-row blocks per dma_gather call
    mm_bf16: bool  # bf16 tables/matmul operands (accumulation stays f32)

    @property
    def shard(self) -> int:
        return -(-self.n_nodes // self.n_cores)

    @property
    def tiles(self) -> int:
        return -(-self.shard // P)

    @property
    def shard_pad(self) -> int:
        return self.tiles * P

    @property
    def bsz1(self) -> int:  # layer-1 table bucket size (x: n_nodes rows)
        return -(-self.n_nodes // NBUCK)

    @property
    def bsz2(self) -> int:  # layer-2 table bucket size (h_full rows)
        return -(-(self.n_cores * self.shard_pad) // NBUCK)


def preprocess(edge_index: np.ndarray, g: Geom):
    """Route edges to dst-owning cores; bucket by src range; pack into blocks.

    Returns (per_core, layout) where per_core[i] has gidx1/gidx2 (int16,
    [P, NB*8], dma_gather 16-wrapped), dl/v (f32 [P, NB]); layout has
    bpt (list over (bucket,tile) in stream order), calls [(c0, k, bucket)].
    """
    n, c, shard, tiles = g.n_nodes, g.n_cores, g.shard, g.tiles
    assert g.bsz1 <= 32768 and g.bsz2 <= 32768
    loops = np.arange(n, dtype=np.int64)
    src = np.concatenate([edge_index[0].astype(np.int64), loops])
    dst = np.concatenate([edge_index[1].astype(np.int64), loops])

    deg = np.bincount(dst, minlength=n).astype(np.float32)
    dinv = (1.0 / np.sqrt(deg)).astype(np.float32)  # deg >= 1 (self loops)
    norm = dinv[src] * dinv[dst]

    core = dst // shard
    local = dst - core * shard
    t_idx = local // P
    dl = (local % P).astype(np.float32)
    buck = src // g.bsz1
    src2 = (src // shard) * g.shard_pad + (src % shard)

    # stream order per core: bucket-major, then tile
    gkey = (core * NBUCK + buck) * tiles + t_idx
    ngrp = c * NBUCK * tiles
    cnt = np.bincount(gkey, minlength=ngrp).reshape(c, NBUCK, tiles)
    # uniform per-core program: blocks per (bucket, tile) = max over cores, >=1
    bpt_bt = -(-cnt.max(axis=0) // P)  # [NBUCK, tiles]; may be 0 for a bucket
    flat_bpt = bpt_bt.reshape(-1)  # stream order (bucket-major)
    colstart = np.zeros(NBUCK * tiles + 1, dtype=np.int64)
    np.cumsum(flat_bpt, out=colstart[1:])
    nb = int(colstart[-1])

    order = np.argsort(gkey, kind="stable")
    gs = np.zeros(ngrp + 1, dtype=np.int64)
    np.cumsum(np.bincount(gkey, minlength=ngrp), out=gs[1:])
    pos = np.arange(len(gkey), dtype=np.int64) - gs[gkey[order]]

    ci = core[order]
    bt_flat = (buck * tiles + t_idx)[order]  # stream group id within core
    slot = pos % P
    column = colstart[bt_flat] + pos // P

    val1 = (src - buck * g.bsz1)[order].astype(np.int16)
    val2 = (src2 - buck * g.bsz2)[order].astype(np.int16)
    assert (src - buck * g.bsz1).max() < 32768 and (src2 - buck * g.bsz2).max() < 32768

    i1 = np.zeros((c, P, nb), dtype=np.int16)
    i2 = np.zeros((c, P, nb), dtype=np.int16)
    dlm = np.zeros((c, P, nb), dtype=np.float32)
    vm = np.zeros((c, P, nb), dtype=np.float32)
    i1[ci, slot, column] = val1
    i2[ci, slot, column] = val2
    dlm[ci, slot, column] = dl[order]
    vm[ci, slot, column] = norm[order]

    # gather calls: chunk each bucket's column range into <=gcols-block calls
    calls = []
    for b in range(NBUCK):
        cs, ce = int(colstart[b * tiles]), int(colstart[(b + 1) * tiles])
        c0 = cs
        while c0 < ce:
            k = min(g.gcols, ce - c0)
            calls.append((c0, k, b))
            c0 += k

    def wrap16(mat):  # [P, nb] msg-block values -> dma_gather idx layout
        out = np.zeros((P, nb * 8), dtype=np.int16)
        for c0, k, _b in calls:
            seg = mat[:, c0 : c0 + k].T.reshape(-1)  # call msgs j = m - c0*128
            w = seg.reshape(k * 8, 16).T  # [16, k*8]
            out[:16, c0 * 8 : (c0 + k) * 8] = w
        return np.tile(out[:16], (8, 1))  # replicate across 8 q7 cores

    per_core = [
        dict(gidx1=wrap16(i1[i]), gidx2=wrap16(i2[i]), dl=dlm[i], v=vm[i])
        for i in range(c)
    ]
    layout = dict(
        bpt_bt=[[int(x) for x in row] for row in bpt_bt],
        colstart=[int(x) for x in colstart],
        calls=calls,
        nb=nb,
    )
    return per_core, layout


def build_program(g: Geom, layout):
    import concourse.bass as bass  # noqa: F401
    import concourse.mybir as mybir
    import concourse.tile as tile
    from concourse import bacc, library_config

    f32 = mybir.dt.float32
    i16 = mybir.dt.int16
    mm_dt = mybir.dt.bfloat16 if g.mm_bf16 else mybir.dt.float32

    nb = layout["nb"]
    bpt_bt = layout["bpt_bt"]
    colstart = layout["colstart"]
    calls = layout["calls"]
    shard, tiles, shard_pad = g.shard, g.tiles, g.shard_pad
    ablate = set(os.environ.get("GCN_ABLATE", "").split(","))  # timing experiments
    stage = os.environ.get("GCN_STAGE", "full")  # g | gs | gsm | full

    # col -> call id
    col2call = np.zeros(nb, dtype=np.int64)
    for ci_, (c0, k, _b) in enumerate(calls):
        col2call[c0 : c0 + k] = ci_

    nc = bacc.Bacc(
        "TRN2",
        target_bir_lowering=False,
        debug=False,
        enable_asserts=False,
        num_devices=g.n_cores,
        num_swdge_queues=4,
        dynamic_dma_scratch_size=int(os.environ.get("GCN_SCRATCH", "65536")),
    )

    x_d = nc.dram_tensor("x", [g.n_nodes, g.in_dim], mm_dt, kind="ExternalInput")
    gi1_d = nc.dram_tensor("gidx1", [P, nb * 8], i16, kind="ExternalInput")
    gi2_d = nc.dram_tensor("gidx2", [P, nb * 8], i16, kind="ExternalInput")
    dl_d = nc.dram_tensor("dl", [P, nb], f32, kind="ExternalInput")
    v_d = nc.dram_tensor("v", [P, nb], f32, kind="ExternalInput")
    w1_d = nc.dram_tensor("w1", [g.in_dim, g.h1], mm_dt, kind="ExternalInput")
    w2_d = nc.dram_tensor("w2", [g.h1, g.h2], mm_dt, kind="ExternalInput")
    b1_d = nc.dram_tensor("b1", [g.h1], f32, kind="ExternalInput")
    b2_d = nc.dram_tensor("b2", [g.h2], f32, kind="ExternalInput")
    io_d = nc.dram_tensor("iotam", [P, P], f32, kind="ExternalInput")
    idm_d = nc.dram_tensor("identm", [P, P], mm_dt, kind="ExternalInput")
    idf_d = nc.dram_tensor("identf", [P, P], f32, kind="ExternalInput")
    out_d = nc.dram_tensor("out", [shard, g.h2], f32, kind="ExternalOutput")

    hb_d = nc.dram_tensor("h_bounce", [shard_pad, g.h1], mm_dt, kind="Internal")
    _shared = "Local" if os.environ.get("GCN_NOSHARED", "0") == "1" else "Shared"
    hf_d = nc.dram_tensor(
        "h_full",
        [g.n_cores * shard_pad, g.h1],
        mm_dt,
        kind="Internal",
        addr_space=_shared,
    )

    with tile.TileContext(nc) as tc:
        with (
            tc.tile_pool(name="const", bufs=1) as cpool,
            tc.tile_pool(name="msg", bufs=int(os.environ.get("GCN_MBUFS", "8"))) as mpool,
            tc.tile_pool(name="sel", bufs=6) as spool,
            tc.tile_pool(name="act", bufs=3) as apool,
            tc.tile_pool(name="psum", bufs=2, space="PSUM") as ppool,
        ):
            nc.gpsimd.load_library(library_config.mlp)

            iota_f = cpool.tile([P, P], f32, tag="iota_f")
            nc.sync.dma_start(out=iota_f[:], in_=io_d[:, :])
            ident = cpool.tile([P, P], mm_dt, tag="ident")
            nc.sync.dma_start(out=ident[:], in_=idm_d[:, :])
            ident_f = cpool.tile([P, P], f32, tag="ident_f")
            nc.sync.dma_start(out=ident_f[:], in_=idf_d[:, :])

            w1_s = cpool.tile([g.in_dim, g.h1], mm_dt, tag="w1")
            nc.sync.dma_start(out=w1_s[:], in_=w1_d[:, :])
            w2_s = cpool.tile([g.h1, g.h2], mm_dt, tag="w2")
            nc.sync.dma_start(out=w2_s[:], in_=w2_d[:, :])
            b1_s = cpool.tile([g.h1, 1], f32, tag="b1")
            nc.sync.dma_start(out=b1_s[:], in_=b1_d[:, None])
            b2_s = cpool.tile([g.h2, 1], f32, tag="b2")
            nc.sync.dma_start(out=b2_s[:], in_=b2_d[:, None])

            gidx_s = cpool.tile([P, nb * 8], i16, tag="gidx")
            dl_s = cpool.tile([P, nb], f32, tag="dl")
            nc.sync.dma_start(out=dl_s[:], in_=dl_d[:, :])
            v_s = cpool.tile([P, nb], f32, tag="v")
            nc.sync.dma_start(out=v_s[:], in_=v_d[:, :])

            def layer(gi_dram, table_bucket_ap, feat, w_s, outw, bias_s, out_dt, store):
                nc.sync.dma_start(out=gidx_s[:], in_=gi_dram[:, :])
                msg_tiles: dict[int, object] = {}

                def ensure_call(ci_: int):
                    if ci_ in msg_tiles:
                        return
                    c0, k, b = calls[ci_]
                    mt = mpool.tile([P, g.gcols * feat], mm_dt, tag="msg")
                    if "gather" in ablate:
                        msg_tiles[ci_] = mt
                        return
                    nc.gpsimd.dma_gather(
                        queue_num=ci_ % 4,
                        out_ap=mt[:].rearrange("p (k d) -> p k d", d=feat)[:, :k, :],
                        in_ap=table_bucket_ap(b),
                        idxs_ap=gidx_s[:, c0 * 8 : (c0 + k) * 8],
                        num_idxs=k * P,
                        num_idxs_reg=k * P,
                        elem_size=feat,
                        # single_packet=True wedges the exec unit above
                        # ~1024 indices per call (HW-probed)
                        single_packet=False,
                    )
                    msg_tiles[ci_] = mt

                for t in range(tiles):
                    blocks = [
                        (b, colstart[b * tiles + t] + blk)
                        for b in range(NBUCK)
                        for blk in range(bpt_bt[b][t])
                    ]
                    assert blocks, f"tile {t} has no message blocks"
                    if stage in ("gsm", "full"):
                        p1 = ppool.tile([P, P], f32, tag="p1", space="PSUM")
                    for i_, (b, col) in enumerate(blocks):
                        ci_ = int(col2call[col])
                        ensure_call(ci_)
                        if ci_ + 1 < len(calls) and col - calls[ci_][0] >= calls[ci_][1] - 3:
                            ensure_call(ci_ + 1)
                        if stage == "g":
                            continue
                        off = col - calls[ci_][0]
                        s_t = spool.tile([P, P], mm_dt, tag="S")
                        nc.vector.tensor_scalar(
                            s_t[:],
                            iota_f[:],
                            dl_s[:, col : col + 1],
                            v_s[:, col : col + 1],
                            op0=mybir.AluOpType.is_equal,
                            op1=mybir.AluOpType.mult,
                        )
                        if stage == "gs":
                            continue
                        nc.tensor.matmul(
                            p1[:],
                            lhsT=msg_tiles[ci_][:, off * feat : (off + 1) * feat],
                            rhs=s_t[:],
                            start=(i_ == 0),
                            stop=(i_ == len(blocks) - 1),
                        )
                    if stage in ("g", "gs", "gsm"):
                        continue
                    a1 = apool.tile([P, P], mm_dt, tag="a1")
                    nc.vector.tensor_copy(a1[:feat, :], p1[:feat, :])
                    p2 = ppool.tile([P, P], f32, tag="p2", space="PSUM")
                    nc.tensor.matmul(
                        p2[:outw, :], lhsT=w_s[:feat, :outw], rhs=a1[:feat, :],
                        start=True, stop=True,
                    )
                    ht = apool.tile([P, P], out_dt, tag="ht")
                    nc.scalar.activation(
                        ht[:outw, :], p2[:outw, :],
                        mybir.ActivationFunctionType.Relu,
                        bias=bias_s[:outw, :],
                    )
                    pt = ppool.tile([P, P], out_dt, tag="pt", space="PSUM")
                    idn = ident if out_dt == mm_dt else ident_f
                    nc.tensor.transpose(
                        pt[:, :outw], ht[:outw, :], idn[:outw, :outw]
                    )
                    hrow = apool.tile([P, P], out_dt, tag="hrow")
                    nc.vector.tensor_copy(hrow[:, :outw], pt[:, :outw])
                    store(t, hrow)

            def store_l1(t, hrow):
                nc.sync.dma_start(out=hb_d[t * P : (t + 1) * P, :], in_=hrow[:, : g.h1])

            def store_l2(t, hrow):
                rows = min(P, shard - t * P)
                nc.sync.dma_start(
                    out=out_d[t * P : t * P + rows, :], in_=hrow[:rows, : g.h2]
                )

            def tab1(b):
                lo = b * g.bsz1
                hi = min(g.n_nodes, lo + g.bsz1)
                return x_d[lo:hi, :]

            def tab2(b):
                lo = b * g.bsz2
                hi = min(g.n_cores * shard_pad, lo + g.bsz2)
                return hf_d[lo:hi, :]

            layer(gi1_d, tab1, g.in_dim, w1_s, g.h1, b1_s, mm_dt, store_l1)

            tc.strict_bb_all_engine_barrier()
            if os.environ.get("GCN_NOCC", "0") == "1":  # debug: skip collective
                nc.sync.dma_start(out=hf_d[:shard_pad, :], in_=hb_d[:, :])
            else:
                # bf16 AllGather was observed to wedge the exec unit at
                # >=512KB per rank; it is pure data movement, so ship the
                # same bytes as f32.
                cc_in = hb_d.ap() if not g.mm_bf16 else hb_d.ap().bitcast(f32)
                cc_out = hf_d.ap() if not g.mm_bf16 else hf_d.ap().bitcast(f32)
                nc.gpsimd.collective_compute(
                    "AllGather",
                    mybir.AluOpType.bypass,
                    replica_groups=[list(range(g.n_cores))],
                    ins=[cc_in.opt()],
                    outs=[cc_out.opt()],
                )
            tc.strict_bb_all_engine_barrier()

            layer(gi2_d, tab2, g.h1, w2_s, g.h2, b2_s, f32, store_l2)

    nc.compile()
    return nc


_PROGRAM_CACHE: dict = {}
LAST_RESULTS = None  # BassKernelResults of the most recent kernel() call


def _layout_key(layout):
    return (
        tuple(tuple(r) for r in layout["bpt_bt"]),
        tuple(layout["calls"]),
    )


def _get_program(g: Geom, layout):
    key = (g, _layout_key(layout))
    if key not in _PROGRAM_CACHE:
        _PROGRAM_CACHE[key] = build_program(g, layout)
    return _PROGRAM_CACHE[key]


def host_consts(g: Geom):
    import ml_dtypes

    tdt = ml_dtypes.bfloat16 if g.mm_bf16 else np.float32
    iotam = np.tile(np.arange(P, dtype=np.float32), (P, 1))
    ident = np.eye(P, dtype=np.float32)
    return dict(iotam=iotam, identm=ident.astype(tdt), identf=ident)


def run(x, edge_index, W1, b1, W2, b2, g: Geom, trace: bool = False):
    global LAST_RESULTS
    import ml_dtypes
    from concourse.bass_utils import run_bass_kernel_spmd

    per_core, layout = preprocess(np.asarray(edge_index), g)
    nc = _get_program(g, layout)

    tdt = ml_dtypes.bfloat16 if g.mm_bf16 else np.float32
    consts = host_consts(g)
    x_t = np.ascontiguousarray(np.asarray(x)).astype(tdt)
    w1_t = np.asarray(W1).astype(tdt)
    w2_t = np.asarray(W2).astype(tdt)
    b1_t = np.asarray(b1).astype(np.float32)
    b2_t = np.asarray(b2).astype(np.float32)

    in_maps = [
        dict(
            x=x_t, gidx1=pc["gidx1"], gidx2=pc["gidx2"], dl=pc["dl"], v=pc["v"],
            w1=w1_t, w2=w2_t, b1=b1_t, b2=b2_t, **consts,
        )
        for pc in per_core
    ]

    core_ids = list(range(g.n_cores))
    if trace:
        try:
            res = run_bass_kernel_spmd(
                nc, in_maps, core_ids=core_ids, trace=True, trace_cores=[0]
            )
        except Exception as e:  # fall back to an untraced run
            print(f"[kernel] traced run failed ({type(e).__name__}: {e}); retrying untraced")
            res = run_bass_kernel_spmd(nc, in_maps, core_ids=core_ids)
    else:
        res = run_bass_kernel_spmd(nc, in_maps, core_ids=core_ids)
    LAST_RESULTS = res
    out = np.concatenate([r["out"] for r in res.results], axis=0)
    return out[: g.n_nodes]


_FULL = Geom(
    n_nodes=100000,
    n_cores=8,
    in_dim=128,
    h1=128,
    h2=64,
    gcols=int(os.environ.get("GCN_GCOLS", "24")),
    mm_bf16=os.environ.get("GCN_F32", "0") != "1",
)


def kernel(x, edge_index, W1, b1, W2, b2):
    trace = os.environ.get("GCN_TRACE", "0") == "1"
    return run(x, edge_index, W1, b1, W2, b2, _FULL, trace=trace)



# revision 7
# speedup vs baseline: 1.4464x; 1.4464x over previous
"""GCN (2-layer, PyG-style gcn_norm) Bass/Tile kernel for Trainium2, 8 NeuronCores.

Strategy (dst-partitioned message passing, pair-packed gather):
  - Nodes are partitioned across 8 cores by destination; every edge is routed to
    the core that owns its destination node. Self-loops + symmetric D^-1/2 norm
    are computed on the host (index/routing preprocessing only).
  - Gather tables are PAIR-PACKED: two consecutive node rows form one table row
    (layer 1: x viewed [50000, 256] bf16 = 512B rows; layer 2: h@W2 viewed
    [50176, 128] bf16 = 256B rows). One dma_gather descriptor fetches the pair
    row containing a message's source; the parity picks the half. Pair rows
    halve the bucket count (int16 index reach) to 2, cutting block padding and
    descriptor count by ~1/3 vs 4-bucket single-row tables — descriptors are
    the scarce resource (SWDGE Q7 generates them serially at ~8 ns each).
  - Per (bucket, dst-tile) group, messages are packed into 128-slot blocks
    (max-over-cores padded so all 8 cores run one program).
  - The segmented scatter-add becomes TensorE matmuls: per block one DVE
    tensor_scalar builds S_cat[e, 0:256] = (iota256 == dl'_e) * norm_e where
    dl' = dst_local + 128*parity; the two 128-wide halves of S_cat are the
    selection matrices for even/odd-parity messages, consumed by two matmuls
    (lhsT = the matching feature half of the pair row). All-bf16 operands keep
    the DVE op in the 4x perf mode.
  - Layer 1 epilogue: W1 matmul, bias+ReLU (ScalarE), then the W2 matmul is
    FOLDED IN before the halo exchange, so the AllGather ships h@W2 (64 wide,
    1.6 MB/rank) instead of h (128 wide) — half the collective bytes — and
    layer 2 needs no weight multiply at all.
  - Between layers, an AllGather shares the per-core h@W2 shards; layer 2
    scatters gathered pair rows the same way and finishes with bias+ReLU.
"""

import os
from dataclasses import dataclass

import numpy as np

P = 128
NBUCK = 2  # pair-row buckets (int16 gather indices => <=32768 pair rows each)


@dataclass(frozen=True)
class Geom:
    n_nodes: int
    n_cores: int
    in_dim: int
    h1: int
    h2: int
    gcols1: int  # layer-1 gather-group size, in 128-slot blocks per call
    gcols2: int  # layer-2 gather-group size
    mm_bf16: bool  # bf16 tables/matmul operands (accumulation stays f32)

    @property
    def shard(self) -> int:
        return -(-self.n_nodes // self.n_cores)

    @property
    def tiles(self) -> int:
        return -(-self.shard // P)

    @property
    def shard_pad(self) -> int:
        return self.tiles * P

    @property
    def bsz1(self) -> int:  # layer-1 bucket size in PAIR rows (x: n_nodes/2)
        return -(-(self.n_nodes // 2) // NBUCK)

    @property
    def bsz2(self) -> int:  # layer-2 bucket size in PAIR rows (h2w table)
        return -(-(self.n_cores * self.shard_pad // 2) // NBUCK)


def _mk_calls(colstart, tiles, gcols):
    calls = []
    for b in range(NBUCK):
        cs, ce = int(colstart[b * tiles]), int(colstart[(b + 1) * tiles])
        c0 = cs
        while c0 < ce:
            k = min(gcols, ce - c0)
            calls.append((c0, k, b))
            c0 += k
    return calls


def _wrap16(mat, calls, nb):  # [P, nb] slot values -> dma_gather idx layout
    out = np.zeros((P, nb * 8), dtype=np.int16)
    for c0, k, _b in calls:
        seg = mat[:, c0 : c0 + k].T.reshape(-1)  # call msgs j = m - c0*128
        w = seg.reshape(k * 8, 16).T  # [16, k*8]
        out[:16, c0 * 8 : (c0 + k) * 8] = w
    return np.tile(out[:16], (8, 1))  # replicate across 8 q7 cores


def preprocess(edge_index: np.ndarray, g: Geom):
    """Route edges to dst-owning cores; bucket by pair-row range; pack blocks.

    Returns (per_core, layout): per_core[i] has gidx1/gidx2 (int16 [P, NB*8],
    dma_gather 16-wrapped, per-layer call chunking), dlp/v (f32 [P, NB]);
    layout has bpt_bt, colstart, calls1/calls2, nb.
    """
    n, c, shard, tiles = g.n_nodes, g.n_cores, g.shard, g.tiles
    assert g.bsz1 <= 32768 and g.bsz2 <= 32768
    loops = np.arange(n, dtype=np.int64)
    src = np.concatenate([edge_index[0].astype(np.int64), loops])
    dst = np.concatenate([edge_index[1].astype(np.int64), loops])

    deg = np.bincount(dst, minlength=n).astype(np.float32)
    dinv = (1.0 / np.sqrt(deg)).astype(np.float32)  # deg >= 1 (self loops)
    norm = dinv[src] * dinv[dst]

    core = dst // shard
    local = dst - core * shard
    t_idx = local // P
    par = (src % 2).astype(np.float32)  # == src2 % 2 (core*44 is even)
    dlp = (local % P).astype(np.float32) + P * par
    pr1 = src // 2
    buck = pr1 // g.bsz1  # == pair-row-2 bucket (both split at core 3|4)
    src2 = (src // shard) * g.shard_pad + (src % shard)
    pr2 = src2 // 2
    assert (buck == pr2 // g.bsz2).all()

    # stream order per core: bucket-major, then tile
    gkey = (core * NBUCK + buck) * tiles + t_idx
    ngrp = c * NBUCK * tiles
    cnt = np.bincount(gkey, minlength=ngrp).reshape(c, NBUCK, tiles)
    # uniform per-core program: blocks per (bucket, tile) = max over cores, >=1
    bpt_bt = -(-cnt.max(axis=0) // P)  # [NBUCK, tiles]
    flat_bpt = bpt_bt.reshape(-1)  # stream order (bucket-major)
    colstart = np.zeros(NBUCK * tiles + 1, dtype=np.int64)
    np.cumsum(flat_bpt, out=colstart[1:])
    nb = int(colstart[-1])

    order = np.argsort(gkey, kind="stable")
    gs = np.zeros(ngrp + 1, dtype=np.int64)
    np.cumsum(np.bincount(gkey, minlength=ngrp), out=gs[1:])
    pos = np.arange(len(gkey), dtype=np.int64) - gs[gkey[order]]

    ci = core[order]
    bt_flat = (buck * tiles + t_idx)[order]  # stream group id within core
    slot = pos % P
    column = colstart[bt_flat] + pos // P

    val1 = (pr1 - buck * g.bsz1)[order].astype(np.int16)
    val2 = (pr2 - buck * g.bsz2)[order].astype(np.int16)
    assert (pr1 - buck * g.bsz1).max() < 32768 and (pr2 - buck * g.bsz2).max() < 32768

    i1 = np.zeros((c, P, nb), dtype=np.int16)
    i2 = np.zeros((c, P, nb), dtype=np.int16)
    dlm = np.zeros((c, P, nb), dtype=np.float32)
    vm = np.zeros((c, P, nb), dtype=np.float32)
    i1[ci, slot, column] = val1
    i2[ci, slot, column] = val2
    dlm[ci, slot, column] = dlp[order]
    vm[ci, slot, column] = norm[order]

    calls1 = _mk_calls(colstart, tiles, g.gcols1)
    calls2 = _mk_calls(colstart, tiles, g.gcols2)

    per_core = [
        dict(
            gidx1=_wrap16(i1[i], calls1, nb),
            gidx2=_wrap16(i2[i], calls2, nb),
            dl=dlm[i],
            v=vm[i],
        )
        for i in range(c)
    ]
    layout = dict(
        bpt_bt=[[int(x) for x in row] for row in bpt_bt],
        colstart=[int(x) for x in colstart],
        calls1=calls1,
        calls2=calls2,
        nb=nb,
    )
    return per_core, layout


def build_program(g: Geom, layout):
    import concourse.bass as bass  # noqa: F401
    import concourse.mybir as mybir
    import concourse.tile as tile
    from concourse import bacc, library_config

    f32 = mybir.dt.float32
    i16 = mybir.dt.int16
    mm_dt = mybir.dt.bfloat16 if g.mm_bf16 else mybir.dt.float32

    nb = layout["nb"]
    bpt_bt = layout["bpt_bt"]
    colstart = layout["colstart"]
    shard, tiles, shard_pad = g.shard, g.tiles, g.shard_pad
    ablate = set(os.environ.get("GCN_ABLATE", "").split(","))  # timing experiments
    stage = os.environ.get("GCN_STAGE", "full")  # g | gs | gsm | full

    npair1 = g.n_nodes // 2
    npair2 = g.n_cores * shard_pad // 2
    elem1 = 2 * g.in_dim  # pair-row width, elements (512B bf16)
    elem2 = 2 * g.h2  # 128 elements (256B bf16)
    mt_free = g.gcols1 * elem1
    assert g.gcols2 * elem2 <= mt_free

    def col2call_map(calls):
        m = np.zeros(nb, dtype=np.int64)
        for ci_, (c0, k, _b) in enumerate(calls):
            m[c0 : c0 + k] = ci_
        return m

    nc = bacc.Bacc(
        "TRN2",
        target_bir_lowering=False,
        debug=False,
        enable_asserts=False,
        num_devices=g.n_cores,
        num_swdge_queues=4,
        dynamic_dma_scratch_size=int(os.environ.get("GCN_SCRATCH", "65536")),
    )

    x_d = nc.dram_tensor("x", [npair1, elem1], mm_dt, kind="ExternalInput")
    gi1_d = nc.dram_tensor("gidx1", [P, nb * 8], i16, kind="ExternalInput")
    gi2_d = nc.dram_tensor("gidx2", [P, nb * 8], i16, kind="ExternalInput")
    dl_d = nc.dram_tensor("dl", [P, nb], f32, kind="ExternalInput")
    v_d = nc.dram_tensor("v", [P, nb], f32, kind="ExternalInput")
    w1_d = nc.dram_tensor("w1", [g.in_dim, g.h1], mm_dt, kind="ExternalInput")
    w2_d = nc.dram_tensor("w2", [g.h1, g.h2], mm_dt, kind="ExternalInput")
    b1_d = nc.dram_tensor("b1", [g.h1], f32, kind="ExternalInput")
    b2_d = nc.dram_tensor("b2", [g.h2], f32, kind="ExternalInput")
    io_d = nc.dram_tensor("iotam", [P, 2 * P], mm_dt, kind="ExternalInput")
    idm_d = nc.dram_tensor("identm", [P, P], mm_dt, kind="ExternalInput")
    idf_d = nc.dram_tensor("identf", [P, P], f32, kind="ExternalInput")
    out_d = nc.dram_tensor("out", [shard, g.h2], f32, kind="ExternalOutput")

    hb_d = nc.dram_tensor("h_bounce", [shard_pad, g.h2], mm_dt, kind="Internal")
    hf_d = nc.dram_tensor(
        "h_full", [npair2, elem2], mm_dt, kind="Internal", addr_space="Shared"
    )

    with tile.TileContext(nc) as tc:
        with (
            tc.tile_pool(name="const", bufs=1) as cpool,
            tc.tile_pool(name="msg", bufs=int(os.environ.get("GCN_MBUFS", "6"))) as mpool,
            tc.tile_pool(name="sel", bufs=int(os.environ.get("GCN_SBUFS", "8"))) as spool,
            tc.tile_pool(name="act", bufs=4) as apool,
            tc.tile_pool(name="psum", bufs=2, space="PSUM") as ppool,
        ):
            nc.gpsimd.load_library(library_config.mlp)

            iota2 = cpool.tile([P, 2 * P], mm_dt, tag="iota2")
            nc.sync.dma_start(out=iota2[:], in_=io_d[:, :])
            ident = cpool.tile([P, P], mm_dt, tag="ident")
            nc.sync.dma_start(out=ident[:], in_=idm_d[:, :])
            ident_f = cpool.tile([P, P], f32, tag="ident_f")
            nc.sync.dma_start(out=ident_f[:], in_=idf_d[:, :])

            w1_s = cpool.tile([g.in_dim, g.h1], mm_dt, tag="w1")
            nc.sync.dma_start(out=w1_s[:], in_=w1_d[:, :])
            w2_s = cpool.tile([g.h1, g.h2], mm_dt, tag="w2")
            nc.sync.dma_start(out=w2_s[:], in_=w2_d[:, :])
            b1_s = cpool.tile([g.h1, 1], f32, tag="b1")
            nc.sync.dma_start(out=b1_s[:], in_=b1_d[:, None])
            b2_s = cpool.tile([g.h2, 1], f32, tag="b2")
            nc.sync.dma_start(out=b2_s[:], in_=b2_d[:, None])

            gidx_s = cpool.tile([P, nb * 8], i16, tag="gidx")
            dl_s = cpool.tile([P, nb], f32, tag="dl")
            nc.sync.dma_start(out=dl_s[:], in_=dl_d[:, :])
            v_s = cpool.tile([P, nb], f32, tag="v")
            nc.sync.dma_start(out=v_s[:], in_=v_d[:, :])

            def layer(gi_dram, table_bucket_ap, calls, gcols, elem, fh, epilogue):
                # fh: per-node feature width (half the pair-row width)
                col2call = col2call_map(calls)
                nc.sync.dma_start(out=gidx_s[:], in_=gi_dram[:, :])
                msg_tiles: dict[int, object] = {}

                def ensure_call(ci_: int):
                    if ci_ in msg_tiles:
                        return
                    c0, k, b = calls[ci_]
                    mt = mpool.tile([P, mt_free], mm_dt, tag="msg")
                    if "gather" in ablate:
                        msg_tiles[ci_] = mt
                        return
                    nc.gpsimd.dma_gather(
                        queue_num=ci_ % 4,
                        out_ap=mt[:].rearrange("p (k d) -> p k d", d=elem)[:, :k, :],
                        in_ap=table_bucket_ap(b),
                        idxs_ap=gidx_s[:, c0 * 8 : (c0 + k) * 8],
                        num_idxs=k * P,
                        num_idxs_reg=k * P,
                        elem_size=elem,
                        # single_packet=True wedges the exec unit above
                        # ~1024 indices per call (HW-probed)
                        single_packet=os.environ.get("GCN_SP", "0") == "1",
                    )
                    msg_tiles[ci_] = mt

                for t in range(tiles):
                    blocks = [
                        (b, colstart[b * tiles + t] + blk)
                        for b in range(NBUCK)
                        for blk in range(bpt_bt[b][t])
                    ]
                    assert blocks, f"tile {t} has no message blocks"
                    if stage in ("gsm", "full"):
                        p1 = ppool.tile([P, P], f32, tag="p1", space="PSUM")
                    for i_, (b, col) in enumerate(blocks):
                        ci_ = int(col2call[col])
                        ensure_call(ci_)
                        if ci_ + 1 < len(calls) and col - calls[ci_][0] >= calls[ci_][1] - 3:
                            ensure_call(ci_ + 1)
                        if stage == "g":
                            continue
                        off = col - calls[ci_][0]
                        s_t = spool.tile([P, 2 * P], mm_dt, tag="S")
                        nc.vector.tensor_scalar(
                            s_t[:],
                            iota2[:],
                            dl_s[:, col : col + 1],
                            v_s[:, col : col + 1],
                            op0=mybir.AluOpType.is_equal,
                            op1=mybir.AluOpType.mult,
                        )
                        if stage == "gs":
                            continue
                        mt = msg_tiles[ci_]
                        nc.tensor.matmul(
                            p1[:fh, :],
                            lhsT=mt[:, off * elem : off * elem + fh],
                            rhs=s_t[:, 0:P],
                            start=(i_ == 0),
                            stop=False,
                        )
                        nc.tensor.matmul(
                            p1[:fh, :],
                            lhsT=mt[:, off * elem + fh : off * elem + 2 * fh],
                            rhs=s_t[:, P : 2 * P],
                            start=False,
                            stop=(i_ == len(blocks) - 1),
                        )
                    if stage in ("g", "gs", "gsm"):
                        continue
                    epilogue(t, p1)

            def epi_l1(t, p1):
                # p1[:128, :] = aggregated x^T for this dst tile (feat x node)
                a1 = apool.tile([P, P], mm_dt, tag="a1")
                nc.vector.tensor_copy(a1[:, :], p1[:, :])
                p2 = ppool.tile([P, P], f32, tag="p2", space="PSUM")
                nc.tensor.matmul(
                    p2[:, :], lhsT=w1_s[:, :], rhs=a1[:, :], start=True, stop=True
                )
                ht = apool.tile([P, P], mm_dt, tag="ht")
                nc.scalar.activation(
                    ht[:, :], p2[:, :],
                    mybir.ActivationFunctionType.Relu,
                    bias=b1_s[:, :],
                )
                # fold W2 in before the halo exchange: ship h@W2 (64 wide)
                p2b = ppool.tile([P, P], f32, tag="p2b", space="PSUM")
                nc.tensor.matmul(
                    p2b[: g.h2, :], lhsT=w2_s[:, : g.h2], rhs=ht[:, :],
                    start=True, stop=True,
                )
                h2t = apool.tile([P, P], f32, tag="h2t")
                nc.vector.tensor_copy(h2t[: g.h2, :], p2b[: g.h2, :])
                pt = ppool.tile([P, P], f32, tag="pt", space="PSUM")
                nc.tensor.transpose(pt[:, : g.h2], h2t[: g.h2, :], ident_f[: g.h2, : g.h2])
                hrow = apool.tile([P, P], mm_dt, tag="hrow")
                nc.vector.tensor_copy(hrow[:, : g.h2], pt[:, : g.h2])
                nc.sync.dma_start(out=hb_d[t * P : (t + 1) * P, :], in_=hrow[:, : g.h2])

            def epi_l2(t, p1):
                # p1[:64, :] = aggregated (h@W2)^T: just bias+ReLU+store
                ht = apool.tile([P, P], f32, tag="htf")
                nc.scalar.activation(
                    ht[: g.h2, :], p1[: g.h2, :],
                    mybir.ActivationFunctionType.Relu,
                    bias=b2_s[:, :],
                )
                pt = ppool.tile([P, P], f32, tag="pt", space="PSUM")
                nc.tensor.transpose(pt[:, : g.h2], ht[: g.h2, :], ident_f[: g.h2, : g.h2])
                hrow = apool.tile([P, P], f32, tag="hrowf")
                nc.vector.tensor_copy(hrow[:, : g.h2], pt[:, : g.h2])
                rows = min(P, shard - t * P)
                nc.sync.dma_start(
                    out=out_d[t * P : t * P + rows, :], in_=hrow[:rows, : g.h2]
                )

            def tab1(b):
                lo = b * g.bsz1
                hi = min(npair1, lo + g.bsz1)
                return x_d[lo:hi, :]

            def tab2(b):
                lo = b * g.bsz2
                hi = min(npair2, lo + g.bsz2)
                return hf_d[lo:hi, :]

            layer(gi1_d, tab1, layout["calls1"], g.gcols1, elem1, g.in_dim, epi_l1)

            tc.strict_bb_all_engine_barrier()
            if os.environ.get("GCN_NOCC", "0") == "1":  # debug: skip collective
                nc.sync.dma_start(
                    out=hf_d[: shard_pad // 2, :],
                    in_=hb_d[:, :].rearrange("(a b) c -> a (b c)", b=2),
                )
            else:
                # bf16 AllGather was observed to wedge the exec unit at
                # >=512KB per rank; it is pure data movement, so ship the
                # same bytes as f32.
                cc_in = hb_d.ap() if not g.mm_bf16 else hb_d.ap().bitcast(f32)
                cc_out = hf_d.ap() if not g.mm_bf16 else hf_d.ap().bitcast(f32)
                nc.gpsimd.collective_compute(
                    "AllGather",
                    mybir.AluOpType.bypass,
                    replica_groups=[list(range(g.n_cores))],
                    ins=[cc_in.opt()],
                    outs=[cc_out.opt()],
                )
            tc.strict_bb_all_engine_barrier()

            layer(gi2_d, tab2, layout["calls2"], g.gcols2, elem2, g.h2, epi_l2)

    nc.compile()
    return nc


_PROGRAM_CACHE: dict = {}
LAST_RESULTS = None  # BassKernelResults of the most recent kernel() call


def _layout_key(layout):
    return (
        tuple(tuple(r) for r in layout["bpt_bt"]),
        tuple(layout["calls1"]),
        tuple(layout["calls2"]),
    )


def _get_program(g: Geom, layout):
    key = (g, _layout_key(layout))
    if key not in _PROGRAM_CACHE:
        _PROGRAM_CACHE[key] = build_program(g, layout)
    return _PROGRAM_CACHE[key]


def host_consts(g: Geom):
    import ml_dtypes

    tdt = ml_dtypes.bfloat16 if g.mm_bf16 else np.float32
    iotam = np.tile(np.arange(2 * P, dtype=np.float32), (P, 1)).astype(tdt)
    ident = np.eye(P, dtype=np.float32)
    return dict(iotam=iotam, identm=ident.astype(tdt), identf=ident)


def run(x, edge_index, W1, b1, W2, b2, g: Geom, trace: bool = False):
    global LAST_RESULTS
    import ml_dtypes
    from concourse.bass_utils import run_bass_kernel_spmd

    per_core, layout = preprocess(np.asarray(edge_index), g)
    nc = _get_program(g, layout)

    tdt = ml_dtypes.bfloat16 if g.mm_bf16 else np.float32
    consts = host_consts(g)
    x_t = np.ascontiguousarray(np.asarray(x)).astype(tdt).reshape(
        g.n_nodes // 2, 2 * g.in_dim
    )
    w1_t = np.asarray(W1).astype(tdt)
    w2_t = np.asarray(W2).astype(tdt)
    b1_t = np.asarray(b1).astype(np.float32)
    b2_t = np.asarray(b2).astype(np.float32)

    in_maps = [
        dict(
            x=x_t, gidx1=pc["gidx1"], gidx2=pc["gidx2"], dl=pc["dl"], v=pc["v"],
            w1=w1_t, w2=w2_t, b1=b1_t, b2=b2_t, **consts,
        )
        for pc in per_core
    ]

    core_ids = list(range(g.n_cores))
    if trace:
        try:
            res = run_bass_kernel_spmd(
                nc, in_maps, core_ids=core_ids, trace=True, trace_cores=[0]
            )
        except Exception as e:  # fall back to an untraced run
            print(f"[kernel] traced run failed ({type(e).__name__}: {e}); retrying untraced")
            res = run_bass_kernel_spmd(nc, in_maps, core_ids=core_ids)
    else:
        res = run_bass_kernel_spmd(nc, in_maps, core_ids=core_ids)
    LAST_RESULTS = res
    out = np.concatenate([r["out"] for r in res.results], axis=0)
    return out[: g.n_nodes]


_FULL = Geom(
    n_nodes=100000,
    n_cores=8,
    in_dim=128,
    h1=128,
    h2=64,
    gcols1=int(os.environ.get("GCN_GCOLS1", "24")),
    gcols2=int(os.environ.get("GCN_GCOLS2", "48")),
    mm_bf16=os.environ.get("GCN_F32", "0") != "1",
)


def kernel(x, edge_index, W1, b1, W2, b2):
    trace = os.environ.get("GCN_TRACE", "0") == "1"
    return run(x, edge_index, W1, b1, W2, b2, _FULL, trace=trace)


# revision 9
# speedup vs baseline: 2.0842x; 1.4410x over previous
"""GCN (2-layer, PyG-style gcn_norm) Bass/Tile kernel for Trainium2, 8 NeuronCores.

Strategy (dst-partitioned message passing, pair-packed gather):
  - Nodes are partitioned across 8 cores by destination; every edge is routed to
    the core that owns its destination node. Self-loops + symmetric D^-1/2 norm
    are computed on the host (index/routing preprocessing only).
  - Gather tables are PAIR-PACKED: two consecutive node rows form one table row
    (layer 1: x viewed [50000, 256] bf16 = 512B rows; layer 2: h@W2 viewed
    [50176, 128] bf16 = 256B rows). One dma_gather descriptor fetches the pair
    row containing a message's source; the parity picks the half. Pair rows
    halve the bucket count (int16 index reach) to 2, cutting block padding and
    descriptor count by ~1/3 vs 4-bucket single-row tables — descriptors are
    the scarce resource (SWDGE Q7 generates them serially at ~8 ns each).
  - Per (bucket, dst-tile) group, messages are packed into 128-slot blocks
    (max-over-cores padded so all 8 cores run one program).
  - The segmented scatter-add becomes TensorE matmuls: per block one DVE
    tensor_scalar builds S_cat[e, 0:256] = (iota256 == dl'_e) * norm_e where
    dl' = dst_local + 128*parity; the two 128-wide halves of S_cat are the
    selection matrices for even/odd-parity messages, consumed by two matmuls
    (lhsT = the matching feature half of the pair row). All-bf16 operands keep
    the DVE op in the 4x perf mode.
  - Layer 1 epilogue: W1 matmul, bias+ReLU (ScalarE), then the W2 matmul is
    FOLDED IN before the halo exchange, so the AllGather ships h@W2 (64 wide,
    1.6 MB/rank) instead of h (128 wide) — half the collective bytes — and
    layer 2 needs no weight multiply at all.
  - Between layers, an AllGather shares the per-core h@W2 shards; layer 2
    scatters gathered pair rows the same way and finishes with bias+ReLU.
"""

import os
from dataclasses import dataclass

import numpy as np

P = 128
NBUCK = 2  # pair-row buckets (int16 gather indices => <=32768 pair rows each)


@dataclass(frozen=True)
class Geom:
    n_nodes: int
    n_cores: int
    in_dim: int
    h1: int
    h2: int
    gcols1: int  # layer-1 gather-group size, in 128-slot blocks per call
    gcols2: int  # layer-2 gather-group size
    mm_bf16: bool  # bf16 tables/matmul operands (accumulation stays f32)

    @property
    def shard(self) -> int:
        return -(-self.n_nodes // self.n_cores)

    @property
    def tiles(self) -> int:
        return -(-self.shard // P)

    @property
    def shard_pad(self) -> int:
        return self.tiles * P

    @property
    def bsz1(self) -> int:  # layer-1 bucket size in PAIR rows (x: n_nodes/2)
        return -(-(self.n_nodes // 2) // NBUCK)

    @property
    def bsz2(self) -> int:  # layer-2 bucket size in PAIR rows (h2w table)
        return -(-(self.n_cores * self.shard_pad // 2) // NBUCK)


def _mk_calls(colstart, tiles, gcols):
    calls = []
    for b in range(NBUCK):
        cs, ce = int(colstart[b * tiles]), int(colstart[(b + 1) * tiles])
        c0 = cs
        while c0 < ce:
            k = min(gcols, ce - c0)
            calls.append((c0, k, b))
            c0 += k
    return calls


def _wrap16(mat, calls, nb):  # [P, nb] slot values -> dma_gather idx layout
    out = np.zeros((P, nb * 8), dtype=np.int16)
    for c0, k, _b in calls:
        seg = mat[:, c0 : c0 + k].T.reshape(-1)  # call msgs j = m - c0*128
        w = seg.reshape(k * 8, 16).T  # [16, k*8]
        out[:16, c0 * 8 : (c0 + k) * 8] = w
    return np.tile(out[:16], (8, 1))  # replicate across 8 q7 cores


def preprocess(edge_index: np.ndarray, g: Geom):
    """Route edges to dst-owning cores; bucket by pair-row range; pack blocks.

    Returns (per_core, layout): per_core[i] has gidx1/gidx2 (int16 [P, NB*8],
    dma_gather 16-wrapped, per-layer call chunking), dlp/v (f32 [P, NB]);
    layout has bpt_bt, colstart, calls1/calls2, nb.
    """
    n, c, shard, tiles = g.n_nodes, g.n_cores, g.shard, g.tiles
    assert g.bsz1 <= 32768 and g.bsz2 <= 32768
    loops = np.arange(n, dtype=np.int64)
    src = np.concatenate([edge_index[0].astype(np.int64), loops])
    dst = np.concatenate([edge_index[1].astype(np.int64), loops])

    deg = np.bincount(dst, minlength=n).astype(np.float32)
    dinv = (1.0 / np.sqrt(deg)).astype(np.float32)  # deg >= 1 (self loops)
    norm = dinv[src] * dinv[dst]

    core = dst // shard
    local = dst - core * shard
    t_idx = local // P
    par = (src % 2).astype(np.float32)  # == src2 % 2 (core*44 is even)
    dlp = (local % P).astype(np.float32) + P * par
    pr1 = src // 2
    buck = pr1 // g.bsz1  # == pair-row-2 bucket (both split at core 3|4)
    src2 = (src // shard) * g.shard_pad + (src % shard)
    pr2 = src2 // 2
    assert (buck == pr2 // g.bsz2).all()

    # stream order per core: bucket-major, then tile
    gkey = (core * NBUCK + buck) * tiles + t_idx
    ngrp = c * NBUCK * tiles
    cnt = np.bincount(gkey, minlength=ngrp).reshape(c, NBUCK, tiles)
    # uniform per-core program: blocks per (bucket, tile) = max over cores, >=1
    bpt_bt = -(-cnt.max(axis=0) // P)  # [NBUCK, tiles]
    flat_bpt = bpt_bt.reshape(-1)  # stream order (bucket-major)
    colstart = np.zeros(NBUCK * tiles + 1, dtype=np.int64)
    np.cumsum(flat_bpt, out=colstart[1:])
    nb = int(colstart[-1])

    order = np.argsort(gkey, kind="stable")
    gs = np.zeros(ngrp + 1, dtype=np.int64)
    np.cumsum(np.bincount(gkey, minlength=ngrp), out=gs[1:])
    pos = np.arange(len(gkey), dtype=np.int64) - gs[gkey[order]]

    ci = core[order]
    bt_flat = (buck * tiles + t_idx)[order]  # stream group id within core
    slot = pos % P
    column = colstart[bt_flat] + pos // P

    val1 = (pr1 - buck * g.bsz1)[order].astype(np.int16)
    val2 = (pr2 - buck * g.bsz2)[order].astype(np.int16)
    assert (pr1 - buck * g.bsz1).max() < 32768 and (pr2 - buck * g.bsz2).max() < 32768

    i1 = np.zeros((c, P, nb), dtype=np.int16)
    i2 = np.zeros((c, P, nb), dtype=np.int16)
    dlm = np.zeros((c, P, nb), dtype=np.float32)
    vm = np.zeros((c, P, nb), dtype=np.float32)
    i1[ci, slot, column] = val1
    i2[ci, slot, column] = val2
    dlm[ci, slot, column] = dlp[order]
    vm[ci, slot, column] = norm[order]

    calls1 = _mk_calls(colstart, tiles, g.gcols1)
    calls2 = _mk_calls(colstart, tiles, g.gcols2)

    per_core = [
        dict(
            gidx1=_wrap16(i1[i], calls1, nb),
            gidx2=_wrap16(i2[i], calls2, nb),
            dl=dlm[i],
            v=vm[i],
        )
        for i in range(c)
    ]
    layout = dict(
        bpt_bt=[[int(x) for x in row] for row in bpt_bt],
        colstart=[int(x) for x in colstart],
        calls1=calls1,
        calls2=calls2,
        nb=nb,
    )
    return per_core, layout


def build_program(g: Geom, layout):
    import concourse.bass as bass  # noqa: F401
    import concourse.mybir as mybir
    import concourse.tile as tile
    from concourse import bacc, library_config

    f32 = mybir.dt.float32
    i16 = mybir.dt.int16
    mm_dt = mybir.dt.bfloat16 if g.mm_bf16 else mybir.dt.float32

    nb = layout["nb"]
    bpt_bt = layout["bpt_bt"]
    colstart = layout["colstart"]
    shard, tiles, shard_pad = g.shard, g.tiles, g.shard_pad
    ablate = set(os.environ.get("GCN_ABLATE", "").split(","))  # timing experiments
    stage = os.environ.get("GCN_STAGE", "full")  # g | gs | gsm | full

    npair1 = g.n_nodes // 2
    npair2 = g.n_cores * shard_pad // 2
    elem1 = 2 * g.in_dim  # pair-row width, elements (512B bf16)
    elem2 = 2 * g.h2  # 128 elements (256B bf16)
    mt_free = g.gcols1 * elem1
    assert g.gcols2 * elem2 <= mt_free

    def col2call_map(calls):
        m = np.zeros(nb, dtype=np.int64)
        for ci_, (c0, k, _b) in enumerate(calls):
            m[c0 : c0 + k] = ci_
        return m

    nc = bacc.Bacc(
        "TRN2",
        target_bir_lowering=False,
        debug=False,
        enable_asserts=False,
        num_devices=g.n_cores,
        num_swdge_queues=int(os.environ.get("GCN_NQ", "4")),
        dynamic_dma_scratch_size=int(os.environ.get("GCN_SCRATCH", "65536")),
    )

    x_d = nc.dram_tensor("x", [npair1, elem1], mm_dt, kind="ExternalInput")
    gi1_d = nc.dram_tensor("gidx1", [P, nb * 8], i16, kind="ExternalInput")
    gi2_d = nc.dram_tensor("gidx2", [P, nb * 8], i16, kind="ExternalInput")
    dl_d = nc.dram_tensor("dl", [P, nb], f32, kind="ExternalInput")
    v_d = nc.dram_tensor("v", [P, nb], f32, kind="ExternalInput")
    w1_d = nc.dram_tensor("w1", [g.in_dim, g.h1], mm_dt, kind="ExternalInput")
    w2_d = nc.dram_tensor("w2", [g.h1, g.h2], mm_dt, kind="ExternalInput")
    b1_d = nc.dram_tensor("b1", [g.h1], f32, kind="ExternalInput")
    b2_d = nc.dram_tensor("b2", [g.h2], f32, kind="ExternalInput")
    io_d = nc.dram_tensor("iotam", [P, 2 * P], mm_dt, kind="ExternalInput")
    idm_d = nc.dram_tensor("identm", [P, P], mm_dt, kind="ExternalInput")
    idf_d = nc.dram_tensor("identf", [P, P], f32, kind="ExternalInput")
    out_d = nc.dram_tensor("out", [shard, g.h2], f32, kind="ExternalOutput")

    hb_d = nc.dram_tensor("h_bounce", [shard_pad, g.h2], mm_dt, kind="Internal")
    hf_d = nc.dram_tensor(
        "h_full", [npair2, elem2], mm_dt, kind="Internal", addr_space="Shared"
    )

    with tile.TileContext(nc) as tc:
        with (
            tc.tile_pool(name="const", bufs=1) as cpool,
            tc.tile_pool(name="msg", bufs=int(os.environ.get("GCN_MBUFS", "6"))) as mpool,
            tc.tile_pool(name="sel", bufs=int(os.environ.get("GCN_SBUFS", "8"))) as spool,
            tc.tile_pool(name="act", bufs=4) as apool,
            tc.tile_pool(name="psum", bufs=2, space="PSUM") as ppool,
        ):
            nc.gpsimd.load_library(library_config.mlp)

            iota2 = cpool.tile([P, 2 * P], mm_dt, tag="iota2")
            nc.sync.dma_start(out=iota2[:], in_=io_d[:, :])
            ident = cpool.tile([P, P], mm_dt, tag="ident")
            nc.sync.dma_start(out=ident[:], in_=idm_d[:, :])
            ident_f = cpool.tile([P, P], f32, tag="ident_f")
            nc.sync.dma_start(out=ident_f[:], in_=idf_d[:, :])

            w1_s = cpool.tile([g.in_dim, g.h1], mm_dt, tag="w1")
            nc.sync.dma_start(out=w1_s[:], in_=w1_d[:, :])
            w2_s = cpool.tile([g.h1, g.h2], mm_dt, tag="w2")
            nc.sync.dma_start(out=w2_s[:], in_=w2_d[:, :])
            b1_s = cpool.tile([g.h1, 1], f32, tag="b1")
            nc.sync.dma_start(out=b1_s[:], in_=b1_d[:, None])
            b2_s = cpool.tile([g.h2, 1], f32, tag="b2")
            nc.sync.dma_start(out=b2_s[:], in_=b2_d[:, None])

            gidx_s = cpool.tile([P, nb * 8], i16, tag="gidx")
            dl_s = cpool.tile([P, nb], f32, tag="dl")
            nc.sync.dma_start(out=dl_s[:], in_=dl_d[:, :])
            v_s = cpool.tile([P, nb], f32, tag="v")
            nc.sync.dma_start(out=v_s[:], in_=v_d[:, :])

            def layer(gi_dram, table_bucket_ap, calls, gcols, elem, fh, epilogue):
                # fh: per-node feature width (half the pair-row width)
                col2call = col2call_map(calls)
                nc.sync.dma_start(out=gidx_s[:], in_=gi_dram[:, :])
                msg_tiles: dict[int, object] = {}

                def ensure_call(ci_: int):
                    if ci_ in msg_tiles:
                        return
                    c0, k, b = calls[ci_]
                    mt = mpool.tile([P, mt_free], mm_dt, tag="msg")
                    if "gather" in ablate:
                        msg_tiles[ci_] = mt
                        return
                    nc.gpsimd.dma_gather(
                        queue_num=ci_ % int(os.environ.get("GCN_NQ", "4")),
                        out_ap=mt[:].rearrange("p (k d) -> p k d", d=elem)[:, :k, :],
                        in_ap=table_bucket_ap(b),
                        idxs_ap=gidx_s[:, c0 * 8 : (c0 + k) * 8],
                        num_idxs=k * P,
                        num_idxs_reg=k * P,
                        elem_size=elem,
                        # single_packet=True wedges the exec unit above
                        # ~1024 indices per call (HW-probed)
                        single_packet=os.environ.get("GCN_SP", "0") == "1",
                    )
                    msg_tiles[ci_] = mt

                for t in range(tiles):
                    blocks = [
                        (b, colstart[b * tiles + t] + blk)
                        for b in range(NBUCK)
                        for blk in range(bpt_bt[b][t])
                    ]
                    assert blocks, f"tile {t} has no message blocks"
                    if stage in ("gsm", "full"):
                        p1 = ppool.tile([P, P], f32, tag="p1", space="PSUM")
                    for i_, (b, col) in enumerate(blocks):
                        ci_ = int(col2call[col])
                        ensure_call(ci_)
                        if ci_ + 1 < len(calls) and col - calls[ci_][0] >= calls[ci_][1] - 3:
                            ensure_call(ci_ + 1)
                        if stage == "g":
                            continue
                        off = col - calls[ci_][0]
                        s_t = spool.tile([P, 2 * P], mm_dt, tag="S")
                        nc.vector.tensor_scalar(
                            s_t[:],
                            iota2[:],
                            dl_s[:, col : col + 1],
                            v_s[:, col : col + 1],
                            op0=mybir.AluOpType.is_equal,
                            op1=mybir.AluOpType.mult,
                        )
                        if stage == "gs":
                            continue
                        mt = msg_tiles[ci_]
                        nc.tensor.matmul(
                            p1[:fh, :],
                            lhsT=mt[:, off * elem : off * elem + fh],
                            rhs=s_t[:, 0:P],
                            start=(i_ == 0),
                            stop=False,
                        )
                        nc.tensor.matmul(
                            p1[:fh, :],
                            lhsT=mt[:, off * elem + fh : off * elem + 2 * fh],
                            rhs=s_t[:, P : 2 * P],
                            start=False,
                            stop=(i_ == len(blocks) - 1),
                        )
                    if stage in ("g", "gs", "gsm"):
                        continue
                    epilogue(t, p1)

            def epi_l1(t, p1):
                # p1[:128, :] = aggregated x^T for this dst tile (feat x node)
                a1 = apool.tile([P, P], mm_dt, tag="a1")
                nc.vector.tensor_copy(a1[:, :], p1[:, :])
                p2 = ppool.tile([P, P], f32, tag="p2", space="PSUM")
                nc.tensor.matmul(
                    p2[:, :], lhsT=w1_s[:, :], rhs=a1[:, :], start=True, stop=True
                )
                ht = apool.tile([P, P], mm_dt, tag="ht")
                nc.scalar.activation(
                    ht[:, :], p2[:, :],
                    mybir.ActivationFunctionType.Relu,
                    bias=b1_s[:, :],
                )
                # fold W2 in before the halo exchange: ship h@W2 (64 wide)
                p2b = ppool.tile([P, P], f32, tag="p2b", space="PSUM")
                nc.tensor.matmul(
                    p2b[: g.h2, :], lhsT=w2_s[:, : g.h2], rhs=ht[:, :],
                    start=True, stop=True,
                )
                h2t = apool.tile([P, P], f32, tag="h2t")
                nc.vector.tensor_copy(h2t[: g.h2, :], p2b[: g.h2, :])
                pt = ppool.tile([P, P], f32, tag="pt", space="PSUM")
                nc.tensor.transpose(pt[:, : g.h2], h2t[: g.h2, :], ident_f[: g.h2, : g.h2])
                hrow = apool.tile([P, P], mm_dt, tag="hrow")
                nc.vector.tensor_copy(hrow[:, : g.h2], pt[:, : g.h2])
                nc.sync.dma_start(out=hb_d[t * P : (t + 1) * P, :], in_=hrow[:, : g.h2])

            def epi_l2(t, p1):
                # p1[:64, :] = aggregated (h@W2)^T: just bias+ReLU+store
                ht = apool.tile([P, P], f32, tag="htf")
                nc.scalar.activation(
                    ht[: g.h2, :], p1[: g.h2, :],
                    mybir.ActivationFunctionType.Relu,
                    bias=b2_s[:, :],
                )
                pt = ppool.tile([P, P], f32, tag="pt", space="PSUM")
                nc.tensor.transpose(pt[:, : g.h2], ht[: g.h2, :], ident_f[: g.h2, : g.h2])
                hrow = apool.tile([P, P], f32, tag="hrowf")
                nc.vector.tensor_copy(hrow[:, : g.h2], pt[:, : g.h2])
                rows = min(P, shard - t * P)
                nc.sync.dma_start(
                    out=out_d[t * P : t * P + rows, :], in_=hrow[:rows, : g.h2]
                )

            def tab1(b):
                lo = b * g.bsz1
                hi = min(npair1, lo + g.bsz1)
                return x_d[lo:hi, :]

            def tab2(b):
                lo = b * g.bsz2
                hi = min(npair2, lo + g.bsz2)
                return hf_d[lo:hi, :]

            layer(gi1_d, tab1, layout["calls1"], g.gcols1, elem1, g.in_dim, epi_l1)

            tc.strict_bb_all_engine_barrier()
            if os.environ.get("GCN_NOCC", "0") == "1":  # debug: skip collective
                nc.sync.dma_start(
                    out=hf_d[: shard_pad // 2, :],
                    in_=hb_d[:, :].rearrange("(a b) c -> a (b c)", b=2),
                )
            else:
                # bf16 AllGather was observed to wedge the exec unit at
                # >=512KB per rank; it is pure data movement, so ship the
                # same bytes as f32.
                cc_in = hb_d.ap() if not g.mm_bf16 else hb_d.ap().bitcast(f32)
                cc_out = hf_d.ap() if not g.mm_bf16 else hf_d.ap().bitcast(f32)
                nc.gpsimd.collective_compute(
                    "AllGather",
                    mybir.AluOpType.bypass,
                    replica_groups=[list(range(g.n_cores))],
                    ins=[cc_in.opt()],
                    outs=[cc_out.opt()],
                )
            tc.strict_bb_all_engine_barrier()

            layer(gi2_d, tab2, layout["calls2"], g.gcols2, elem2, g.h2, epi_l2)

    nc.compile()
    return nc


_PROGRAM_CACHE: dict = {}
LAST_RESULTS = None  # BassKernelResults of the most recent kernel() call


def _layout_key(layout):
    return (
        tuple(tuple(r) for r in layout["bpt_bt"]),
        tuple(layout["calls1"]),
        tuple(layout["calls2"]),
    )


def _get_program(g: Geom, layout):
    key = (g, _layout_key(layout))
    if key not in _PROGRAM_CACHE:
        _PROGRAM_CACHE[key] = build_program(g, layout)
    return _PROGRAM_CACHE[key]


def host_consts(g: Geom):
    import ml_dtypes

    tdt = ml_dtypes.bfloat16 if g.mm_bf16 else np.float32
    iotam = np.tile(np.arange(2 * P, dtype=np.float32), (P, 1)).astype(tdt)
    ident = np.eye(P, dtype=np.float32)
    return dict(iotam=iotam, identm=ident.astype(tdt), identf=ident)


def run(x, edge_index, W1, b1, W2, b2, g: Geom, trace: bool = False):
    global LAST_RESULTS
    import ml_dtypes
    from concourse.bass_utils import run_bass_kernel_spmd

    per_core, layout = preprocess(np.asarray(edge_index), g)
    nc = _get_program(g, layout)

    tdt = ml_dtypes.bfloat16 if g.mm_bf16 else np.float32
    consts = host_consts(g)
    x_t = np.ascontiguousarray(np.asarray(x)).astype(tdt).reshape(
        g.n_nodes // 2, 2 * g.in_dim
    )
    w1_t = np.asarray(W1).astype(tdt)
    w2_t = np.asarray(W2).astype(tdt)
    b1_t = np.asarray(b1).astype(np.float32)
    b2_t = np.asarray(b2).astype(np.float32)

    in_maps = [
        dict(
            x=x_t, gidx1=pc["gidx1"], gidx2=pc["gidx2"], dl=pc["dl"], v=pc["v"],
            w1=w1_t, w2=w2_t, b1=b1_t, b2=b2_t, **consts,
        )
        for pc in per_core
    ]

    core_ids = list(range(g.n_cores))
    if trace:
        try:
            res = run_bass_kernel_spmd(
                nc, in_maps, core_ids=core_ids, trace=True, trace_cores=[0]
            )
        except Exception as e:  # fall back to an untraced run
            print(f"[kernel] traced run failed ({type(e).__name__}: {e}); retrying untraced")
            res = run_bass_kernel_spmd(nc, in_maps, core_ids=core_ids)
    else:
        res = run_bass_kernel_spmd(nc, in_maps, core_ids=core_ids)
    LAST_RESULTS = res
    out = np.concatenate([r["out"] for r in res.results], axis=0)
    return out[: g.n_nodes]


_FULL = Geom(
    n_nodes=100000,
    n_cores=8,
    in_dim=128,
    h1=128,
    h2=64,
    gcols1=int(os.environ.get("GCN_GCOLS1", "24")),
    gcols2=int(os.environ.get("GCN_GCOLS2", "48")),
    mm_bf16=os.environ.get("GCN_F32", "0") != "1",
)


def kernel(x, edge_index, W1, b1, W2, b2):
    trace = os.environ.get("GCN_TRACE", "0") == "1"
    return run(x, edge_index, W1, b1, W2, b2, _FULL, trace=trace)


# revision 12
# speedup vs baseline: 2.4429x; 1.1721x over previous
"""GCN (2-layer, PyG-style gcn_norm) Bass/Tile kernel for Trainium2, 8 NeuronCores.

Strategy (dst-partitioned message passing, pair-packed gather, separable norm):
  - Nodes are partitioned across 8 cores by destination; every edge is routed
    to the core that owns its destination node. Self-loop + symmetric
    D^-1/2 A D^-1/2 normalization is computed on the host (index/routing
    preprocessing only).
  - The norm dinv[src]*dinv[dst] is SEPARABLE: dinv[src] is folded into the
    gather tables (x is pre-scaled on the host; the layer-1 epilogue scales
    the h@W2 table rows), and dinv[dst] is applied once per output tile.
    The per-edge selection matrices are then PURE 0/1 one-hots.
  - Gather tables are PAIR-PACKED: two consecutive node rows form one table
    row (layer 1: dinv*x viewed [50000, 256] bf16 = 512B rows; layer 2:
    dinv*(h@W2) viewed [50176, 128] bf16 = 256B rows). One dma_gather
    descriptor fetches the pair row containing a message's source; the parity
    picks the half. Pair rows halve the bucket count (int16 index reach) to
    2, cutting block padding and descriptor count ~1/3 — descriptors are the
    scarce resource (the SWDGE ring drain rate gates the whole kernel).
  - Per (bucket, dst-tile) group, non-self edges are packed into 128-slot
    blocks (max-over-cores padded so all 8 cores run one program). Per block
    one single-op DVE tensor_scalar builds S_cat[e, 0:256] =
    (iota256 == dl'_e) where dl' = dst_local + 128*parity (pad slots use
    dl' = 300 so their one-hot row is all zero); the two 128-wide halves are
    the even/odd selection matrices. Two matmuls per block accumulate
    p1[node, feat] += S_half^T @ msg_half (nodes on PSUM partitions).
  - SELF-LOOPS never enter the gather stream: each tile's 64 own pair rows
    are loaded with cheap sequential HWDGE DMA (from a per-core xown input /
    the local h bounce buffer) and scattered with a CONSTANT [64, 256]
    selection matrix (S_self[i, 2i] = S_self[i, 128+2i+1] = 1).
  - Layer-1 epilogue: dinv[dst] scale, transpose, W1 matmul, bias+ReLU
    (ScalarE), W2 matmul folded in BEFORE the halo exchange (AllGather ships
    h@W2, 64 wide, half the bytes), transpose, dinv scale for the table.
  - Layer-2 bias enters as a K=1 rank-1 matmul (b2 x 1/dinv[dst]) inside the
    PSUM accumulation; the output orientation [node, feat] is already
    row-major, so layer 2 needs no transpose: scale, ReLU, store.
"""

import os
from dataclasses import dataclass

import numpy as np

P = 128
NBUCK = 2  # pair-row buckets (int16 gather indices => <=32768 pair rows each)
PAD_DL = 300.0  # sentinel: one-hot of 300 over iota 0..255 is all-zero


@dataclass(frozen=True)
class Geom:
    n_nodes: int
    n_cores: int
    in_dim: int
    h1: int
    h2: int
    gcols1: int  # layer-1 gather-group size, in 128-slot blocks per call
    gcols2: int  # layer-2 gather-group size
    selfk: int  # tiles per sequential self-row DMA chunk
    mm_bf16: bool  # bf16 tables/matmul operands (accumulation stays f32)

    @property
    def shard(self) -> int:
        return -(-self.n_nodes // self.n_cores)

    @property
    def tiles(self) -> int:
        return -(-self.shard // P)

    @property
    def shard_pad(self) -> int:
        return self.tiles * P

    @property
    def bsz1(self) -> int:  # layer-1 bucket size in PAIR rows
        return -(-(self.n_nodes // 2) // NBUCK)

    @property
    def bsz2(self) -> int:  # layer-2 bucket size in PAIR rows
        return -(-(self.n_cores * self.shard_pad // 2) // NBUCK)


def _mk_calls(colstart, tiles, gcols):
    calls = []
    for b in range(NBUCK):
        cs, ce = int(colstart[b * tiles]), int(colstart[(b + 1) * tiles])
        c0 = cs
        while c0 < ce:
            k = min(gcols, ce - c0)
            calls.append((c0, k, b))
            c0 += k
    return calls


def _wrap16(mat, calls, nb):  # [P, nb] slot values -> dma_gather idx layout
    out = np.zeros((P, nb * 8), dtype=np.int16)
    for c0, k, _b in calls:
        seg = mat[:, c0 : c0 + k].T.reshape(-1)  # call msgs j = m - c0*128
        w = seg.reshape(k * 8, 16).T  # [16, k*8]
        out[:16, c0 * 8 : (c0 + k) * 8] = w
    return np.tile(out[:16], (8, 1))  # replicate across 8 q7 cores


def preprocess(edge_index: np.ndarray, g: Geom):
    """Route edges (no self-loops) to dst cores; bucket by pair row; pack.

    Returns (per_core, layout, dinv): per_core[i] has gidx1/gidx2 (int16
    [P, NB*8]), dl (f32 [P, NB], dst_local + 128*parity, PAD_DL for pads),
    ddst (f32 [P, tiles], dinv of own nodes), sdeg ([1, tiles*P], 1/dinv).
    """
    n, c, shard, tiles = g.n_nodes, g.n_cores, g.shard, g.tiles
    assert g.bsz1 <= 32768 and g.bsz2 <= 32768
    src = edge_index[0].astype(np.int64)
    dst = edge_index[1].astype(np.int64)

    deg = np.bincount(dst, minlength=n).astype(np.float32) + 1.0  # + self loop
    dinv = (1.0 / np.sqrt(deg)).astype(np.float32)

    core = dst // shard
    local = dst - core * shard
    t_idx = local // P
    par = (src % 2).astype(np.float32)  # == src2 % 2 (core*44 is even)
    dlp = (local % P).astype(np.float32) + P * par
    pr1 = src // 2
    buck = pr1 // g.bsz1  # == pair-row-2 bucket (both split at core 3|4)
    src2 = (src // shard) * g.shard_pad + (src % shard)
    pr2 = src2 // 2

    # stream order per core: bucket-major, then tile
    gkey = (core * NBUCK + buck) * tiles + t_idx
    ngrp = c * NBUCK * tiles
    cnt = np.bincount(gkey, minlength=ngrp).reshape(c, NBUCK, tiles)
    # uniform per-core program: blocks per (bucket, tile) = max over cores, >=1
    bpt_bt = -(-cnt.max(axis=0) // P)  # [NBUCK, tiles]
    flat_bpt = bpt_bt.reshape(-1)  # stream order (bucket-major)
    colstart = np.zeros(NBUCK * tiles + 1, dtype=np.int64)
    np.cumsum(flat_bpt, out=colstart[1:])
    nb = int(colstart[-1])

    order = np.argsort(gkey, kind="stable")
    gs = np.zeros(ngrp + 1, dtype=np.int64)
    np.cumsum(np.bincount(gkey, minlength=ngrp), out=gs[1:])
    pos = np.arange(len(gkey), dtype=np.int64) - gs[gkey[order]]

    ci = core[order]
    bt_flat = (buck * tiles + t_idx)[order]  # stream group id within core
    slot = pos % P
    column = colstart[bt_flat] + pos // P

    val1 = (pr1 - buck * g.bsz1)[order].astype(np.int16)
    val2 = (pr2 - buck * g.bsz2)[order].astype(np.int16)
    assert (pr1 - buck * g.bsz1).max() < 32768 and (pr2 - buck * g.bsz2).max() < 32768

    i1 = np.zeros((c, P, nb), dtype=np.int16)
    i2 = np.zeros((c, P, nb), dtype=np.int16)
    dlm = np.full((c, P, nb), PAD_DL, dtype=np.float32)
    i1[ci, slot, column] = val1
    i2[ci, slot, column] = val2
    dlm[ci, slot, column] = dlp[order]

    calls1 = _mk_calls(colstart, tiles, g.gcols1)
    calls2 = _mk_calls(colstart, tiles, g.gcols2)

    dpad = np.zeros(c * g.shard_pad, dtype=np.float32)
    spad = np.ones(c * g.shard_pad, dtype=np.float32)
    for i in range(c):
        lo, hi = i * shard, (i + 1) * shard
        dpad[i * g.shard_pad : i * g.shard_pad + shard] = dinv[lo:hi]
        spad[i * g.shard_pad : i * g.shard_pad + shard] = np.sqrt(deg[lo:hi])

    per_core = [
        dict(
            gidx1=_wrap16(i1[i], calls1, nb),
            gidx2=_wrap16(i2[i], calls2, nb),
            dl=dlm[i],
            ddst=dpad[i * g.shard_pad : (i + 1) * g.shard_pad]
            .reshape(tiles, P)
            .T.copy(),
            sdeg=spad[i * g.shard_pad : (i + 1) * g.shard_pad][None, :].copy(),
        )
        for i in range(c)
    ]
    layout = dict(
        bpt_bt=[[int(x) for x in row] for row in bpt_bt],
        colstart=[int(x) for x in colstart],
        calls1=calls1,
        calls2=calls2,
        nb=nb,
    )
    return per_core, layout, dinv


def build_program(g: Geom, layout):
    import concourse.bass as bass  # noqa: F401
    import concourse.mybir as mybir
    import concourse.tile as tile
    from concourse import bacc, library_config

    f32 = mybir.dt.float32
    i16 = mybir.dt.int16
    mm_dt = mybir.dt.bfloat16 if g.mm_bf16 else mybir.dt.float32

    nb = layout["nb"]
    bpt_bt = layout["bpt_bt"]
    colstart = layout["colstart"]
    shard, tiles, shard_pad = g.shard, g.tiles, g.shard_pad
    ablate = set(os.environ.get("GCN_ABLATE", "").split(","))  # timing experiments
    stage = os.environ.get("GCN_STAGE", "full")  # g | gs | gsm | full
    nq = int(os.environ.get("GCN_NQ", "4"))
    sp = os.environ.get("GCN_SP", "1") == "1"

    npair1 = g.n_nodes // 2
    npair2 = g.n_cores * shard_pad // 2
    elem1 = 2 * g.in_dim  # pair-row width, elements (512B bf16)
    elem2 = 2 * g.h2  # 128 elements (256B bf16)
    mt_free = g.gcols1 * elem1
    assert g.gcols2 * elem2 <= mt_free
    selfk = g.selfk
    nchunk = -(-tiles // selfk)

    def col2call_map(calls):
        m = np.zeros(nb, dtype=np.int64)
        for ci_, (c0, k, _b) in enumerate(calls):
            m[c0 : c0 + k] = ci_
        return m

    nc = bacc.Bacc(
        "TRN2",
        target_bir_lowering=False,
        debug=False,
        enable_asserts=False,
        num_devices=g.n_cores,
        num_swdge_queues=nq,
        dynamic_dma_scratch_size=int(os.environ.get("GCN_SCRATCH", "65536")),
    )

    x_d = nc.dram_tensor("x", [npair1, elem1], mm_dt, kind="ExternalInput")
    xo_d = nc.dram_tensor("xown", [shard_pad // 2, elem1], mm_dt, kind="ExternalInput")
    gi1_d = nc.dram_tensor("gidx1", [P, nb * 8], i16, kind="ExternalInput")
    gi2_d = nc.dram_tensor("gidx2", [P, nb * 8], i16, kind="ExternalInput")
    dl_d = nc.dram_tensor("dl", [P, nb], f32, kind="ExternalInput")
    dd_d = nc.dram_tensor("ddst", [P, tiles], f32, kind="ExternalInput")
    sd_d = nc.dram_tensor("sdeg", [1, tiles * P], f32, kind="ExternalInput")
    w1_d = nc.dram_tensor("w1", [g.in_dim, g.h1], mm_dt, kind="ExternalInput")
    w2_d = nc.dram_tensor("w2", [g.h1, g.h2], mm_dt, kind="ExternalInput")
    b1_d = nc.dram_tensor("b1", [g.h1], f32, kind="ExternalInput")
    b2r_d = nc.dram_tensor("b2row", [1, g.h2], f32, kind="ExternalInput")
    io_d = nc.dram_tensor("iotam", [P, 2 * P], mm_dt, kind="ExternalInput")
    ss_d = nc.dram_tensor("sself", [P // 2, 2 * P], mm_dt, kind="ExternalInput")
    idm_d = nc.dram_tensor("identm", [P, P], mm_dt, kind="ExternalInput")
    out_d = nc.dram_tensor("out", [shard, g.h2], f32, kind="ExternalOutput")

    hb_d = nc.dram_tensor("h_bounce", [shard_pad, g.h2], mm_dt, kind="Internal")
    hf_d = nc.dram_tensor(
        "h_full", [npair2, elem2], mm_dt, kind="Internal", addr_space="Shared"
    )

    with tile.TileContext(nc) as tc:
        with (
            tc.tile_pool(name="const", bufs=1) as cpool,
            tc.tile_pool(name="msg", bufs=int(os.environ.get("GCN_MBUFS", "6"))) as mpool,
            tc.tile_pool(name="selfp", bufs=3) as fpool,
            tc.tile_pool(name="sel", bufs=int(os.environ.get("GCN_SBUFS", "8"))) as spool,
            tc.tile_pool(name="act", bufs=4) as apool,
            tc.tile_pool(name="psum", bufs=2, space="PSUM") as ppool,
        ):
            nc.gpsimd.load_library(library_config.mlp)

            iota2 = cpool.tile([P, 2 * P], mm_dt, tag="iota2")
            nc.sync.dma_start(out=iota2[:], in_=io_d[:, :])
            sself = cpool.tile([P // 2, 2 * P], mm_dt, tag="sself")
            nc.sync.dma_start(out=sself[:], in_=ss_d[:, :])
            ident = cpool.tile([P, P], mm_dt, tag="ident")
            nc.sync.dma_start(out=ident[:], in_=idm_d[:, :])

            w1_s = cpool.tile([g.in_dim, g.h1], mm_dt, tag="w1")
            nc.sync.dma_start(out=w1_s[:], in_=w1_d[:, :])
            w2_s = cpool.tile([g.h1, g.h2], mm_dt, tag="w2")
            nc.sync.dma_start(out=w2_s[:], in_=w2_d[:, :])
            b1_s = cpool.tile([g.h1, 1], f32, tag="b1")
            nc.sync.dma_start(out=b1_s[:], in_=b1_d[:, None])
            b2r_s = cpool.tile([1, g.h2], f32, tag="b2r")
            nc.sync.dma_start(out=b2r_s[:], in_=b2r_d[:, :])
            dd_s = cpool.tile([P, tiles], f32, tag="ddst")
            nc.sync.dma_start(out=dd_s[:], in_=dd_d[:, :])
            sd_s = cpool.tile([1, tiles * P], f32, tag="sdeg")
            nc.sync.dma_start(out=sd_s[:], in_=sd_d[:, :])

            gidx_s = cpool.tile([P, nb * 8], i16, tag="gidx")
            dl_s = cpool.tile([P, nb], f32, tag="dl")
            nc.sync.dma_start(out=dl_s[:], in_=dl_d[:, :])

            def layer(gi_dram, table_bucket_ap, self_chunk_ap, calls, elem, fh,
                      rank1_bias, epilogue):
                col2call = col2call_map(calls)
                nc.sync.dma_start(out=gidx_s[:], in_=gi_dram[:, :])
                msg_tiles: dict[int, object] = {}
                self_tiles: dict[int, object] = {}

                def ensure_call(ci_: int):
                    if ci_ in msg_tiles:
                        return
                    c0, k, b = calls[ci_]
                    mt = mpool.tile([P, mt_free], mm_dt, tag="msg")
                    if "gather" in ablate:
                        msg_tiles[ci_] = mt
                        return
                    nc.gpsimd.dma_gather(
                        queue_num=ci_ % nq,
                        out_ap=mt[:].rearrange("p (k d) -> p k d", d=elem)[:, :k, :],
                        in_ap=table_bucket_ap(b),
                        idxs_ap=gidx_s[:, c0 * 8 : (c0 + k) * 8],
                        num_idxs=k * P,
                        num_idxs_reg=k * P,
                        elem_size=elem,
                        single_packet=sp,
                    )
                    msg_tiles[ci_] = mt

                def ensure_self(ch: int):
                    if ch in self_tiles:
                        return
                    t0 = ch * selfk
                    kk = min(selfk, tiles - t0)
                    ft = fpool.tile([P // 2, selfk * elem1], mm_dt, tag="selfmt")
                    nc.sync.dma_start(
                        out=ft[:, : kk * elem].rearrange("i (t e) -> i t e", e=elem),
                        in_=self_chunk_ap(t0, kk),
                    )
                    self_tiles[ch] = ft

                for t in range(tiles):
                    blocks = [
                        (b, colstart[b * tiles + t] + blk)
                        for b in range(NBUCK)
                        for blk in range(bpt_bt[b][t])
                    ]
                    ensure_self(t // selfk)
                    if t // selfk + 1 < nchunk and t % selfk >= selfk - 2:
                        ensure_self(t // selfk + 1)
                    if stage in ("gsm", "full"):
                        p1 = ppool.tile([P, P], f32, tag="p1", space="PSUM")
                        # rank-1 bias term (layer 2): p1[n, f] = b2[f] / d_n
                        if rank1_bias:
                            nc.tensor.matmul(
                                p1[:, :fh],
                                lhsT=sd_s[:, t * P : (t + 1) * P],
                                rhs=b2r_s[:, :],
                                start=True,
                                stop=False,
                            )
                        # self-loop block: constant one-hot selection
                        ft = self_tiles[t // selfk]
                        so = (t % selfk) * elem
                        nc.tensor.matmul(
                            p1[:, :fh],
                            lhsT=sself[:, 0:P],
                            rhs=ft[:, so : so + fh],
                            start=not rank1_bias,
                            stop=False,
                        )
                        nc.tensor.matmul(
                            p1[:, :fh],
                            lhsT=sself[:, P : 2 * P],
                            rhs=ft[:, so + fh : so + 2 * fh],
                            start=False,
                            stop=False,
                        )
                    for i_, (b, col) in enumerate(blocks):
                        ci_ = int(col2call[col])
                        ensure_call(ci_)
                        if ci_ + 1 < len(calls) and col - calls[ci_][0] >= calls[ci_][1] - 3:
                            ensure_call(ci_ + 1)
                        if stage == "g":
                            continue
                        off = col - calls[ci_][0]
                        s_t = spool.tile([P, 2 * P], mm_dt, tag="S")
                        nc.vector.tensor_scalar(
                            s_t[:],
                            iota2[:],
                            dl_s[:, col : col + 1],
                            None,
                            op0=mybir.AluOpType.is_equal,
                        )
                        if stage == "gs":
                            continue
                        mt = msg_tiles[ci_]
                        nc.tensor.matmul(
                            p1[:, :fh],
                            lhsT=s_t[:, 0:P],
                            rhs=mt[:, off * elem : off * elem + fh],
                            start=False,
                            stop=False,
                        )
                        nc.tensor.matmul(
                            p1[:, :fh],
                            lhsT=s_t[:, P : 2 * P],
                            rhs=mt[:, off * elem + fh : off * elem + 2 * fh],
                            start=False,
                            stop=(i_ == len(blocks) - 1),
                        )
                    if stage in ("g", "gs", "gsm"):
                        continue
                    epilogue(t, p1)

            def epi_l1(t, p1):
                # p1[node, in_dim] aggregated; scale by dinv[dst], cast bf16
                a1 = apool.tile([P, P], mm_dt, tag="a1")
                nc.vector.tensor_scalar(
                    a1[:, :], p1[:, :], dd_s[:, t : t + 1], None,
                    op0=mybir.AluOpType.mult,
                )
                ptr = ppool.tile([P, P], mm_dt, tag="ptr", space="PSUM")
                nc.tensor.transpose(ptr[:, :], a1[:, :], ident[:, :])
                a2 = apool.tile([P, P], mm_dt, tag="a2")
                nc.vector.tensor_copy(a2[:, :], ptr[:, :])
                p2 = ppool.tile([P, P], f32, tag="p2", space="PSUM")
                nc.tensor.matmul(
                    p2[:, :], lhsT=w1_s[:, :], rhs=a2[:, :], start=True, stop=True
                )
                ht = apool.tile([P, P], mm_dt, tag="ht")
                nc.scalar.activation(
                    ht[:, :], p2[:, :],
                    mybir.ActivationFunctionType.Relu,
                    bias=b1_s[:, :],
                )
                p2b = ppool.tile([P, P], f32, tag="p2b", space="PSUM")
                nc.tensor.matmul(
                    p2b[: g.h2, :], lhsT=w2_s[:, : g.h2], rhs=ht[:, :],
                    start=True, stop=True,
                )
                h2t = apool.tile([P, P], mm_dt, tag="h2t")
                nc.vector.tensor_copy(h2t[: g.h2, :], p2b[: g.h2, :])
                pt = ppool.tile([P, P], mm_dt, tag="ptr", space="PSUM")
                nc.tensor.transpose(pt[:, : g.h2], h2t[: g.h2, :], ident[: g.h2, : g.h2])
                hrow = apool.tile([P, P], mm_dt, tag="hrow")
                nc.vector.tensor_scalar(
                    hrow[:, : g.h2], pt[:, : g.h2], dd_s[:, t : t + 1], None,
                    op0=mybir.AluOpType.mult,
                )
                nc.sync.dma_start(out=hb_d[t * P : (t + 1) * P, :], in_=hrow[:, : g.h2])

            def epi_l2(t, p1):
                # p1[node, h2] aggregated (incl. rank-1 bias/d term);
                # out = relu(dinv[dst] * p1) -- already row-major
                a1 = apool.tile([P, P], f32, tag="a1f")
                nc.vector.tensor_scalar(
                    a1[:, : g.h2], p1[:, : g.h2], dd_s[:, t : t + 1], None,
                    op0=mybir.AluOpType.mult,
                )
                hrow = apool.tile([P, P], f32, tag="hrowf")
                nc.scalar.activation(
                    hrow[:, : g.h2], a1[:, : g.h2],
                    mybir.ActivationFunctionType.Relu,
                )
                rows = min(P, shard - t * P)
                nc.sync.dma_start(
                    out=out_d[t * P : t * P + rows, :], in_=hrow[:rows, : g.h2]
                )

            def tab1(b):
                lo = b * g.bsz1
                hi = min(npair1, lo + g.bsz1)
                return x_d[lo:hi, :]

            def tab2(b):
                lo = b * g.bsz2
                hi = min(npair2, lo + g.bsz2)
                return hf_d[lo:hi, :]

            def self1(t0, kk):
                return xo_d[t0 * 64 : (t0 + kk) * 64, :].rearrange(
                    "(t i) e -> i t e", i=64
                )

            def self2(t0, kk):
                return hb_d[t0 * P : (t0 + kk) * P, :].rearrange(
                    "(t i b) c -> i t (b c)", i=64, b=2
                )

            layer(gi1_d, tab1, self1, layout["calls1"], elem1, g.in_dim, False, epi_l1)

            tc.strict_bb_all_engine_barrier()
            if os.environ.get("GCN_NOCC", "0") == "1":  # debug: skip collective
                nc.sync.dma_start(
                    out=hf_d[: shard_pad // 2, :],
                    in_=hb_d[:, :].rearrange("(a b) c -> a (b c)", b=2),
                )
            else:
                # bf16 AllGather was observed to wedge the exec unit at
                # >=512KB per rank; it is pure data movement, so ship the
                # same bytes as f32.
                cc_in = hb_d.ap() if not g.mm_bf16 else hb_d.ap().bitcast(f32)
                cc_out = hf_d.ap() if not g.mm_bf16 else hf_d.ap().bitcast(f32)
                nc.gpsimd.collective_compute(
                    "AllGather",
                    mybir.AluOpType.bypass,
                    replica_groups=[list(range(g.n_cores))],
                    ins=[cc_in.opt()],
                    outs=[cc_out.opt()],
                )
            tc.strict_bb_all_engine_barrier()

            layer(gi2_d, tab2, self2, layout["calls2"], elem2, g.h2, True, epi_l2)

    nc.compile()
    return nc


_PROGRAM_CACHE: dict = {}
LAST_RESULTS = None  # BassKernelResults of the most recent kernel() call


def _layout_key(layout):
    return (
        tuple(tuple(r) for r in layout["bpt_bt"]),
        tuple(layout["calls1"]),
        tuple(layout["calls2"]),
    )


def _get_program(g: Geom, layout):
    key = (g, _layout_key(layout))
    if key not in _PROGRAM_CACHE:
        _PROGRAM_CACHE[key] = build_program(g, layout)
    return _PROGRAM_CACHE[key]


def host_consts(g: Geom):
    import ml_dtypes

    tdt = ml_dtypes.bfloat16 if g.mm_bf16 else np.float32
    iotam = np.tile(np.arange(2 * P, dtype=np.float32), (P, 1)).astype(tdt)
    sself = np.zeros((P // 2, 2 * P), dtype=np.float32)
    for i in range(P // 2):
        sself[i, 2 * i] = 1.0
        sself[i, P + 2 * i + 1] = 1.0
    ident = np.eye(P, dtype=np.float32)
    return dict(iotam=iotam, sself=sself.astype(tdt), identm=ident.astype(tdt))


def run(x, edge_index, W1, b1, W2, b2, g: Geom, trace: bool = False):
    global LAST_RESULTS
    import ml_dtypes
    from concourse.bass_utils import run_bass_kernel_spmd

    per_core, layout, dinv = preprocess(np.asarray(edge_index), g)
    nc = _get_program(g, layout)

    tdt = ml_dtypes.bfloat16 if g.mm_bf16 else np.float32
    consts = host_consts(g)
    xs = np.asarray(x) * dinv[:, None]  # fold dinv[src] into the table
    x_t = np.ascontiguousarray(xs).astype(tdt).reshape(g.n_nodes // 2, 2 * g.in_dim)
    w1_t = np.asarray(W1).astype(tdt)
    w2_t = np.asarray(W2).astype(tdt)
    b1_t = np.asarray(b1).astype(np.float32)
    b2_t = np.asarray(b2).astype(np.float32)[None, :]

    xo_pad = np.zeros((g.shard_pad // 2, 2 * g.in_dim), dtype=tdt)
    in_maps = []
    for i, pc in enumerate(per_core):
        lo = i * g.shard
        xo = xo_pad.copy()
        xo[: g.shard // 2] = x_t[lo // 2 : (lo + g.shard) // 2]
        in_maps.append(
            dict(
                x=x_t, xown=xo, gidx1=pc["gidx1"], gidx2=pc["gidx2"], dl=pc["dl"],
                ddst=pc["ddst"], sdeg=pc["sdeg"], w1=w1_t, w2=w2_t, b1=b1_t,
                b2row=b2_t, **consts,
            )
        )

    core_ids = list(range(g.n_cores))
    if trace:
        try:
            res = run_bass_kernel_spmd(
                nc, in_maps, core_ids=core_ids, trace=True, trace_cores=[0]
            )
        except Exception as e:  # fall back to an untraced run
            print(f"[kernel] traced run failed ({type(e).__name__}: {e}); retrying untraced")
            res = run_bass_kernel_spmd(nc, in_maps, core_ids=core_ids)
    else:
        res = run_bass_kernel_spmd(nc, in_maps, core_ids=core_ids)
    LAST_RESULTS = res
    out = np.concatenate([r["out"] for r in res.results], axis=0)
    return out[: g.n_nodes]


_FULL = Geom(
    n_nodes=100000,
    n_cores=8,
    in_dim=128,
    h1=128,
    h2=64,
    gcols1=int(os.environ.get("GCN_GCOLS1", "7")),
    gcols2=int(os.environ.get("GCN_GCOLS2", "7")),
    selfk=14,
    mm_bf16=os.environ.get("GCN_F32", "0") != "1",
)


def kernel(x, edge_index, W1, b1, W2, b2):
    trace = os.environ.get("GCN_TRACE", "0") == "1"
    return run(x, edge_index, W1, b1, W2, b2, _FULL, trace=trace)


# revision 15
# speedup vs baseline: 2.4834x; 1.0166x over previous
"""GCN (2-layer, PyG-style gcn_norm) Bass/Tile kernel for Trainium2, 8 NeuronCores.

Strategy (dst-partitioned message passing, dense-packed gathers, separable norm):
  - Nodes are partitioned across 8 cores by destination; every edge is routed
    to the core that owns its destination node. Self-loop + symmetric
    D^-1/2 A D^-1/2 normalization is computed on the host (index/routing
    preprocessing only).
  - The norm dinv[src]*dinv[dst] is SEPARABLE: dinv[src] is folded into the
    gather tables (x is pre-scaled on the host; the layer-1 epilogue scales
    the h@W2 table rows), and dinv[dst] is applied once per output tile.
    The per-edge selection matrices are then PURE 0/1 one-hots, built by one
    single-op DVE tensor_scalar per block: S[e, c] = (iota == dl_e), where
    out-of-tile / padding slots carry the sentinel dl = 300 (all-zero row).
  - The SWDGE gather drain is byte-bound, so each layer picks the smallest
    legal descriptor: layer 1 gathers single 256B rows of dinv*x
    ([100000, 128] bf16, 4 int16-reach buckets); layer 2 gathers 256B PAIR
    rows of dinv*(h@W2) ([50176, 128] bf16, 2 buckets) with the parity
    selecting the half (dl' = dst_local + 128*parity, two matmuls per block
    on the S_cat halves).
  - Messages are packed DENSELY: per (bucket, dst-tile) group sized to
    roundup16(max-over-cores count) so all 8 cores run one program; 128-slot
    blocks cut across group boundaries, and a block shared by adjacent tiles
    is matmul'd once per tile with the other tile's slots sentinel-masked.
  - Matmuls accumulate p1[node, feat] += S^T @ msg in PSUM (nodes on
    partitions). Self-loops never enter the gather stream: each tile's own
    rows arrive by cheap sequential HWDGE DMA (from a per-core xown input /
    the local h bounce buffer) and are scattered with constant selection
    matrices (identity for layer 1, a fixed [64, 256] pattern for layer 2).
  - Layer-1 epilogue: dinv[dst] scale, transpose, W1 matmul, bias+ReLU
    (ScalarE), W2 matmul folded in BEFORE the halo exchange (the AllGather
    ships h@W2, 64 wide -- half the bytes), transpose, dinv scale.
  - Layer-2 bias enters as a K=1 rank-1 matmul (b2 x sqrt(deg)) inside the
    PSUM accumulation; the [node, feat] output orientation is already
    row-major, so layer 2 finishes with just scale, ReLU, store.
"""

import os
from dataclasses import dataclass

import numpy as np

P = 128
PAD_DL = 300.0  # sentinel: one-hot of 300 over iota 0..255 is all-zero


@dataclass(frozen=True)
class Geom:
    n_nodes: int
    n_cores: int
    in_dim: int
    h1: int
    h2: int
    gcols1: int  # layer-1 gather-group size, in 128-slot blocks per call
    gcols2: int  # layer-2 gather-group size
    selfk: int  # tiles per sequential self-row DMA chunk
    mm_bf16: bool  # bf16 tables/matmul operands (accumulation stays f32)

    @property
    def shard(self) -> int:
        return -(-self.n_nodes // self.n_cores)

    @property
    def tiles(self) -> int:
        return -(-self.shard // P)

    @property
    def shard_pad(self) -> int:
        return self.tiles * P


def _pack_layer(core, t_idx, dl_vals, row, nbuck, bsz, tiles, n_cores, gcols):
    """Dense pad-16 packing of one layer's messages.

    Returns dict with per-core idx [P, nb*8] (wrap16), dl [P, n_mm] (f32,
    PAD_DL sentinels), plus layout: nb (blocks), calls, sched (per tile:
    list of (block, mm_col)).
    """
    buck = row // bsz
    val = (row - buck * bsz).astype(np.int16)
    assert int(row.max()) - int(buck.max()) * bsz < 32768

    gkey = (core * nbuck + buck) * tiles + t_idx
    ngrp = n_cores * nbuck * tiles
    cnt = np.bincount(gkey, minlength=ngrp).reshape(n_cores, nbuck, tiles)
    size_bt = ((cnt.max(axis=0) + 15) // 16) * 16  # [nbuck, tiles]

    # group slot starts, bucket-major; bucket totals padded to whole blocks
    start_bt = np.zeros((nbuck, tiles), dtype=np.int64)
    off = 0
    bucket_span = []  # (block0, nblocks) per bucket
    for b in range(nbuck):
        blk0 = off // P
        for t in range(tiles):
            start_bt[b, t] = off
            off += int(size_bt[b, t])
        off = ((off + P - 1) // P) * P
        bucket_span.append((blk0, off // P - blk0))
    nb = off // P

    # matmul schedule: per (b, t) the overlapped blocks, t-major mm columns
    sched = [[] for _ in range(tiles)]
    k0_bt = np.zeros((nbuck, tiles), dtype=np.int64)
    jstart_bt = np.zeros((nbuck, tiles), dtype=np.int64)
    j = 0
    for t in range(tiles):
        for b in range(nbuck):
            s, e = int(start_bt[b, t]), int(start_bt[b, t] + size_bt[b, t])
            if e == s:
                continue
            ks = range(s // P, (e + P - 1) // P)
            k0_bt[b, t] = s // P
            jstart_bt[b, t] = j
            for k in ks:
                sched[t].append((k, j))
                j += 1
    n_mm = j

    # place each message: global slot, block, lane, mm column
    order = np.argsort(gkey, kind="stable")
    gs = np.zeros(ngrp + 1, dtype=np.int64)
    np.cumsum(np.bincount(gkey, minlength=ngrp), out=gs[1:])
    pos = np.arange(len(gkey), dtype=np.int64) - gs[gkey[order]]

    ci = core[order]
    b_o, t_o = buck[order], t_idx[order]
    slot = start_bt[b_o, t_o] + pos
    kblk = slot // P
    lane = slot % P
    jcol = jstart_bt[b_o, t_o] + (kblk - k0_bt[b_o, t_o])

    idxm = np.zeros((n_cores, P, nb), dtype=np.int16)
    dlm = np.full((n_cores, P, n_mm), PAD_DL, dtype=np.float32)
    idxm[ci, lane, kblk] = val[order]
    dlm[ci, lane, jcol] = dl_vals[order]

    calls = []
    for blk0, nblk in bucket_span:
        c0 = blk0
        while c0 < blk0 + nblk:
            k = min(gcols, blk0 + nblk - c0)
            calls.append((c0, k, len(calls)))
            c0 += k
    # rewrite third field as bucket id for table slicing
    calls = [
        (c0, k, next(b for b, (b0, nn) in enumerate(bucket_span) if b0 <= c0 < b0 + nn))
        for (c0, k, _x) in calls
    ]

    def wrap16(mat):
        out = np.zeros((P, nb * 8), dtype=np.int16)
        for c0, k, _b in calls:
            seg = mat[:, c0 : c0 + k].T.reshape(-1)
            out[:16, c0 * 8 : (c0 + k) * 8] = seg.reshape(k * 8, 16).T
        return np.tile(out[:16], (8, 1))

    per_core = [dict(idx=wrap16(idxm[i]), dl=dlm[i]) for i in range(n_cores)]
    layout = dict(nb=nb, n_mm=n_mm, calls=calls, sched=sched)
    return per_core, layout


def preprocess(edge_index: np.ndarray, g: Geom):
    n, c, shard, tiles = g.n_nodes, g.n_cores, g.shard, g.tiles
    src = edge_index[0].astype(np.int64)
    dst = edge_index[1].astype(np.int64)

    deg = np.bincount(dst, minlength=n).astype(np.float32) + 1.0  # + self loop
    dinv = (1.0 / np.sqrt(deg)).astype(np.float32)

    core = dst // shard
    local = dst - core * shard
    t_idx = local // P
    dl = (local % P).astype(np.float32)

    # layer 1: single rows of x, 4 buckets
    pc1, lay1 = _pack_layer(
        core, t_idx, dl, src, 4, -(-n // 4), tiles, c, g.gcols1
    )
    # layer 2: pair rows of h@W2, 2 buckets; parity in dl'
    src2 = (src // shard) * g.shard_pad + (src % shard)
    npair2 = c * g.shard_pad // 2
    dlp = dl + P * (src2 % 2).astype(np.float32)
    pc2, lay2 = _pack_layer(
        core, t_idx, dlp, src2 // 2, 2, -(-npair2 // 2), tiles, c, g.gcols2
    )

    dpad = np.zeros(c * g.shard_pad, dtype=np.float32)
    spad = np.ones(c * g.shard_pad, dtype=np.float32)
    for i in range(c):
        lo, hi = i * shard, (i + 1) * shard
        dpad[i * g.shard_pad : i * g.shard_pad + shard] = dinv[lo:hi]
        spad[i * g.shard_pad : i * g.shard_pad + shard] = np.sqrt(deg[lo:hi])

    per_core = [
        dict(
            gidx1=pc1[i]["idx"],
            gidx2=pc2[i]["idx"],
            dl1=pc1[i]["dl"],
            dl2=pc2[i]["dl"],
            ddst=dpad[i * g.shard_pad : (i + 1) * g.shard_pad]
            .reshape(tiles, P)
            .T.copy(),
            sdeg=spad[i * g.shard_pad : (i + 1) * g.shard_pad][None, :].copy(),
        )
        for i in range(c)
    ]
    return per_core, dict(l1=lay1, l2=lay2), dinv


def build_program(g: Geom, layout):
    import concourse.bass as bass  # noqa: F401
    import concourse.mybir as mybir
    import concourse.tile as tile
    from concourse import bacc, library_config

    f32 = mybir.dt.float32
    i16 = mybir.dt.int16
    mm_dt = mybir.dt.bfloat16 if g.mm_bf16 else mybir.dt.float32

    shard, tiles, shard_pad = g.shard, g.tiles, g.shard_pad
    ablate = set(os.environ.get("GCN_ABLATE", "").split(","))  # timing experiments
    stage = os.environ.get("GCN_STAGE", "full")  # g | gs | gsm | full
    nq = int(os.environ.get("GCN_NQ", "4"))
    sp = os.environ.get("GCN_SP", "1") == "1"

    lay1, lay2 = layout["l1"], layout["l2"]
    nb1, nb2 = lay1["nb"], lay2["nb"]
    nmm1, nmm2 = lay1["n_mm"], lay2["n_mm"]
    npair2 = g.n_cores * shard_pad // 2
    elem1 = g.in_dim  # single-row width (256B bf16)
    elem2 = 2 * g.h2  # pair-row width, 128 elements (256B bf16)
    bsz1 = -(-g.n_nodes // 4)
    bsz2 = -(-npair2 // 2)
    mt_free = g.gcols1 * elem1
    assert g.gcols2 * elem2 <= mt_free
    selfk = g.selfk
    nchunk = -(-tiles // selfk)

    nc = bacc.Bacc(
        "TRN2",
        target_bir_lowering=False,
        debug=False,
        enable_asserts=False,
        num_devices=g.n_cores,
        num_swdge_queues=nq,
        dynamic_dma_scratch_size=int(os.environ.get("GCN_SCRATCH", "65536")),
    )

    x_d = nc.dram_tensor("x", [g.n_nodes, elem1], mm_dt, kind="ExternalInput")
    xo_d = nc.dram_tensor("xown", [shard_pad, elem1], mm_dt, kind="ExternalInput")
    gi1_d = nc.dram_tensor("gidx1", [P, nb1 * 8], i16, kind="ExternalInput")
    gi2_d = nc.dram_tensor("gidx2", [P, nb2 * 8], i16, kind="ExternalInput")
    dl1_d = nc.dram_tensor("dl1", [P, nmm1], f32, kind="ExternalInput")
    dl2_d = nc.dram_tensor("dl2", [P, nmm2], f32, kind="ExternalInput")
    dd_d = nc.dram_tensor("ddst", [P, tiles], f32, kind="ExternalInput")
    sd_d = nc.dram_tensor("sdeg", [1, tiles * P], f32, kind="ExternalInput")
    w1_d = nc.dram_tensor("w1", [g.in_dim, g.h1], mm_dt, kind="ExternalInput")
    w2_d = nc.dram_tensor("w2", [g.h1, g.h2], mm_dt, kind="ExternalInput")
    b1_d = nc.dram_tensor("b1", [g.h1], f32, kind="ExternalInput")
    b2r_d = nc.dram_tensor("b2row", [1, g.h2], f32, kind="ExternalInput")
    io_d = nc.dram_tensor("iotam", [P, 2 * P], mm_dt, kind="ExternalInput")
    ss_d = nc.dram_tensor("sself", [P // 2, 2 * P], mm_dt, kind="ExternalInput")
    idm_d = nc.dram_tensor("identm", [P, P], mm_dt, kind="ExternalInput")
    out_d = nc.dram_tensor("out", [shard, g.h2], f32, kind="ExternalOutput")

    hb_d = nc.dram_tensor("h_bounce", [shard_pad, g.h2], mm_dt, kind="Internal")
    hf_d = nc.dram_tensor(
        "h_full", [npair2, elem2], mm_dt, kind="Internal", addr_space="Shared"
    )

    with tile.TileContext(nc) as tc:
        with (
            tc.tile_pool(name="const", bufs=1) as cpool,
            tc.tile_pool(name="msg", bufs=int(os.environ.get("GCN_MBUFS", "8"))) as mpool,
            tc.tile_pool(name="selfp", bufs=3) as fpool,
            tc.tile_pool(name="sel", bufs=int(os.environ.get("GCN_SBUFS", "8"))) as spool,
            tc.tile_pool(name="act", bufs=4) as apool,
            tc.tile_pool(name="psum", bufs=2, space="PSUM") as ppool,
        ):
            nc.gpsimd.load_library(library_config.mlp)

            iota2 = cpool.tile([P, 2 * P], mm_dt, tag="iota2")
            nc.sync.dma_start(out=iota2[:], in_=io_d[:, :])
            sself = cpool.tile([P // 2, 2 * P], mm_dt, tag="sself")
            nc.sync.dma_start(out=sself[:], in_=ss_d[:, :])
            ident = cpool.tile([P, P], mm_dt, tag="ident")
            nc.sync.dma_start(out=ident[:], in_=idm_d[:, :])

            w1_s = cpool.tile([g.in_dim, g.h1], mm_dt, tag="w1")
            nc.sync.dma_start(out=w1_s[:], in_=w1_d[:, :])
            w2_s = cpool.tile([g.h1, g.h2], mm_dt, tag="w2")
            nc.sync.dma_start(out=w2_s[:], in_=w2_d[:, :])
            b1_s = cpool.tile([g.h1, 1], f32, tag="b1")
            nc.sync.dma_start(out=b1_s[:], in_=b1_d[:, None])
            b2r_s = cpool.tile([1, g.h2], f32, tag="b2r")
            nc.sync.dma_start(out=b2r_s[:], in_=b2r_d[:, :])
            dd_s = cpool.tile([P, tiles], f32, tag="ddst")
            nc.sync.dma_start(out=dd_s[:], in_=dd_d[:, :])
            sd_s = cpool.tile([1, tiles * P], f32, tag="sdeg")
            nc.sync.dma_start(out=sd_s[:], in_=sd_d[:, :])

            gidx_s = cpool.tile([P, max(nb1, nb2) * 8], i16, tag="gidx")
            dl_s = cpool.tile([P, max(nmm1, nmm2)], f32, tag="dl")

            def layer(gi_dram, dl_dram, nb, lay, table_bucket_ap, self_chunk_ap,
                      elem, fh, pair, rank1_bias, epilogue):
                calls = lay["calls"]
                sched = lay["sched"]
                col2call = np.zeros(nb, dtype=np.int64)
                for ci_, (c0, k, _b) in enumerate(calls):
                    col2call[c0 : c0 + k] = ci_
                nc.sync.dma_start(out=gidx_s[:, : nb * 8], in_=gi_dram[:, :])
                nc.sync.dma_start(out=dl_s[:, : lay["n_mm"]], in_=dl_dram[:, :])
                msg_tiles: dict[int, object] = {}
                self_tiles: dict[int, object] = {}

                def ensure_call(ci_: int):
                    if ci_ in msg_tiles:
                        return
                    c0, k, b = calls[ci_]
                    mt = mpool.tile([P, mt_free], mm_dt, tag="msg")
                    if "gather" in ablate:
                        msg_tiles[ci_] = mt
                        return
                    nc.gpsimd.dma_gather(
                        queue_num=ci_ % nq,
                        out_ap=mt[:].rearrange("p (k d) -> p k d", d=elem)[:, :k, :],
                        in_ap=table_bucket_ap(b),
                        idxs_ap=gidx_s[:, c0 * 8 : (c0 + k) * 8],
                        num_idxs=k * P,
                        num_idxs_reg=k * P,
                        elem_size=elem,
                        single_packet=sp,
                    )
                    msg_tiles[ci_] = mt

                def ensure_self(ch: int):
                    if ch in self_tiles:
                        return
                    t0 = ch * selfk
                    kk = min(selfk, tiles - t0)
                    prt = P if not pair else P // 2
                    ft = fpool.tile([P, selfk * elem1], mm_dt, tag="selfmt")
                    nc.sync.dma_start(
                        out=ft[:prt, : kk * elem].rearrange(
                            "i (t e) -> i t e", e=elem
                        ),
                        in_=self_chunk_ap(t0, kk),
                    )
                    self_tiles[ch] = ft

                for t in range(tiles):
                    ensure_self(t // selfk)
                    if t // selfk + 1 < nchunk and t % selfk >= selfk - 2:
                        ensure_self(t // selfk + 1)
                    if stage in ("gsm", "full"):
                        p1 = ppool.tile([P, P], f32, tag="p1", space="PSUM")
                        if rank1_bias:
                            nc.tensor.matmul(
                                p1[:, :fh],
                                lhsT=sd_s[:, t * P : (t + 1) * P],
                                rhs=b2r_s[:, :],
                                start=True,
                                stop=False,
                            )
                        ft = self_tiles[t // selfk]
                        so = (t % selfk) * elem
                        if pair:
                            nc.tensor.matmul(
                                p1[:, :fh],
                                lhsT=sself[:, 0:P],
                                rhs=ft[: P // 2, so : so + fh],
                                start=not rank1_bias,
                                stop=False,
                            )
                            nc.tensor.matmul(
                                p1[:, :fh],
                                lhsT=sself[:, P : 2 * P],
                                rhs=ft[: P // 2, so + fh : so + 2 * fh],
                                start=False,
                                stop=False,
                            )
                        else:
                            nc.tensor.matmul(
                                p1[:, :fh],
                                lhsT=ident[:, :],
                                rhs=ft[:, so : so + fh],
                                start=not rank1_bias,
                                stop=False,
                            )
                    for i_, (k, jmm) in enumerate(sched[t]):
                        ci_ = int(col2call[k])
                        ensure_call(ci_)
                        if ci_ + 1 < len(calls) and k - calls[ci_][0] >= calls[ci_][1] - 3:
                            ensure_call(ci_ + 1)
                        if stage == "g":
                            continue
                        off = k - calls[ci_][0]
                        wid = 2 * P if pair else P
                        s_t = spool.tile([P, 2 * P], mm_dt, tag="S")
                        nc.vector.tensor_scalar(
                            s_t[:, :wid],
                            iota2[:, :wid],
                            dl_s[:, jmm : jmm + 1],
                            None,
                            op0=mybir.AluOpType.is_equal,
                        )
                        if stage == "gs":
                            continue
                        mt = msg_tiles[ci_]
                        last = i_ == len(sched[t]) - 1
                        nc.tensor.matmul(
                            p1[:, :fh],
                            lhsT=s_t[:, 0:P],
                            rhs=mt[:, off * elem : off * elem + fh],
                            start=False,
                            stop=last and not pair,
                        )
                        if pair:
                            nc.tensor.matmul(
                                p1[:, :fh],
                                lhsT=s_t[:, P : 2 * P],
                                rhs=mt[:, off * elem + fh : off * elem + 2 * fh],
                                start=False,
                                stop=last,
                            )
                    if stage in ("g", "gs", "gsm"):
                        continue
                    epilogue(t, p1)

            def epi_l1(t, p1):
                # p1[node, in_dim] aggregated; scale by dinv[dst], cast bf16
                a1 = apool.tile([P, P], mm_dt, tag="a1")
                nc.vector.tensor_scalar(
                    a1[:, :], p1[:, :], dd_s[:, t : t + 1], None,
                    op0=mybir.AluOpType.mult,
                )
                ptr = ppool.tile([P, P], mm_dt, tag="ptr", space="PSUM")
                nc.tensor.transpose(ptr[:, :], a1[:, :], ident[:, :])
                a2 = apool.tile([P, P], mm_dt, tag="a2")
                nc.vector.tensor_copy(a2[:, :], ptr[:, :])
                p2 = ppool.tile([P, P], f32, tag="p2", space="PSUM")
                nc.tensor.matmul(
                    p2[:, :], lhsT=w1_s[:, :], rhs=a2[:, :], start=True, stop=True
                )
                ht = apool.tile([P, P], mm_dt, tag="ht")
                nc.scalar.activation(
                    ht[:, :], p2[:, :],
                    mybir.ActivationFunctionType.Relu,
                    bias=b1_s[:, :],
                )
                p2b = ppool.tile([P, P], f32, tag="p2b", space="PSUM")
                nc.tensor.matmul(
                    p2b[: g.h2, :], lhsT=w2_s[:, : g.h2], rhs=ht[:, :],
                    start=True, stop=True,
                )
                h2t = apool.tile([P, P], mm_dt, tag="h2t")
                nc.vector.tensor_copy(h2t[: g.h2, :], p2b[: g.h2, :])
                pt = ppool.tile([P, P], mm_dt, tag="ptr", space="PSUM")
                nc.tensor.transpose(pt[:, : g.h2], h2t[: g.h2, :], ident[: g.h2, : g.h2])
                hrow = apool.tile([P, P], mm_dt, tag="hrow")
                nc.vector.tensor_scalar(
                    hrow[:, : g.h2], pt[:, : g.h2], dd_s[:, t : t + 1], None,
                    op0=mybir.AluOpType.mult,
                )
                nc.sync.dma_start(out=hb_d[t * P : (t + 1) * P, :], in_=hrow[:, : g.h2])

            def epi_l2(t, p1):
                # p1[node, h2] aggregated (incl. rank-1 bias term);
                # out = relu(dinv[dst] * p1) -- already row-major
                hrow = apool.tile([P, P], f32, tag="hrowf")
                nc.scalar.activation(
                    hrow[:, : g.h2], p1[:, : g.h2],
                    mybir.ActivationFunctionType.Relu,
                    scale=dd_s[:, t : t + 1],
                )
                rows = min(P, shard - t * P)
                nc.sync.dma_start(
                    out=out_d[t * P : t * P + rows, :], in_=hrow[:rows, : g.h2]
                )

            def tab1(b):
                lo = b * bsz1
                hi = min(g.n_nodes, lo + bsz1)
                return x_d[lo:hi, :]

            def tab2(b):
                lo = b * bsz2
                hi = min(npair2, lo + bsz2)
                return hf_d[lo:hi, :]

            def self1(t0, kk):
                return xo_d[t0 * P : (t0 + kk) * P, :].rearrange(
                    "(t i) e -> i t e", i=P
                )

            def self2(t0, kk):
                return hb_d[t0 * P : (t0 + kk) * P, :].rearrange(
                    "(t i b) c -> i t (b c)", i=64, b=2
                )

            layer(gi1_d, dl1_d, nb1, lay1, tab1, self1, elem1, g.in_dim, False,
                  False, epi_l1)

            tc.strict_bb_all_engine_barrier()
            if os.environ.get("GCN_NOCC", "0") == "1":  # debug: skip collective
                nc.sync.dma_start(
                    out=hf_d[: shard_pad // 2, :],
                    in_=hb_d[:, :].rearrange("(a b) c -> a (b c)", b=2),
                )
            else:
                # bf16 AllGather was observed to wedge the exec unit at
                # >=512KB per rank; it is pure data movement, so ship the
                # same bytes as f32.
                cc_in = hb_d.ap() if not g.mm_bf16 else hb_d.ap().bitcast(f32)
                cc_out = hf_d.ap() if not g.mm_bf16 else hf_d.ap().bitcast(f32)
                nc.gpsimd.collective_compute(
                    "AllGather",
                    mybir.AluOpType.bypass,
                    replica_groups=[list(range(g.n_cores))],
                    ins=[cc_in.opt()],
                    outs=[cc_out.opt()],
                )
            tc.strict_bb_all_engine_barrier()

            layer(gi2_d, dl2_d, nb2, lay2, tab2, self2, elem2, g.h2, True,
                  True, epi_l2)

    nc.compile()
    return nc


_PROGRAM_CACHE: dict = {}
LAST_RESULTS = None  # BassKernelResults of the most recent kernel() call


def _layout_key(layout):
    def lk(lay):
        return (
            lay["nb"],
            lay["n_mm"],
            tuple(lay["calls"]),
            tuple(tuple(s) for s in lay["sched"]),
        )

    return (lk(layout["l1"]), lk(layout["l2"]))


def _get_program(g: Geom, layout):
    key = (g, _layout_key(layout))
    if key not in _PROGRAM_CACHE:
        _PROGRAM_CACHE[key] = build_program(g, layout)
    return _PROGRAM_CACHE[key]


def host_consts(g: Geom):
    import ml_dtypes

    tdt = ml_dtypes.bfloat16 if g.mm_bf16 else np.float32
    iotam = np.tile(np.arange(2 * P, dtype=np.float32), (P, 1)).astype(tdt)
    sself = np.zeros((P // 2, 2 * P), dtype=np.float32)
    for i in range(P // 2):
        sself[i, 2 * i] = 1.0
        sself[i, P + 2 * i + 1] = 1.0
    ident = np.eye(P, dtype=np.float32)
    return dict(iotam=iotam, sself=sself.astype(tdt), identm=ident.astype(tdt))


def run(x, edge_index, W1, b1, W2, b2, g: Geom, trace: bool = False):
    global LAST_RESULTS
    import ml_dtypes
    from concourse.bass_utils import run_bass_kernel_spmd

    per_core, layout, dinv = preprocess(np.asarray(edge_index), g)
    nc = _get_program(g, layout)

    tdt = ml_dtypes.bfloat16 if g.mm_bf16 else np.float32
    consts = host_consts(g)
    xs = np.asarray(x) * dinv[:, None]  # fold dinv[src] into the table
    x_t = np.ascontiguousarray(xs).astype(tdt)
    w1_t = np.asarray(W1).astype(tdt)
    w2_t = np.asarray(W2).astype(tdt)
    b1_t = np.asarray(b1).astype(np.float32)
    b2_t = np.asarray(b2).astype(np.float32)[None, :]

    xo_pad = np.zeros((g.shard_pad, g.in_dim), dtype=tdt)
    in_maps = []
    for i, pc in enumerate(per_core):
        lo = i * g.shard
        xo = xo_pad.copy()
        xo[: g.shard] = x_t[lo : lo + g.shard]
        in_maps.append(
            dict(
                x=x_t, xown=xo, gidx1=pc["gidx1"], gidx2=pc["gidx2"],
                dl1=pc["dl1"], dl2=pc["dl2"], ddst=pc["ddst"], sdeg=pc["sdeg"],
                w1=w1_t, w2=w2_t, b1=b1_t, b2row=b2_t, **consts,
            )
        )

    core_ids = list(range(g.n_cores))
    if trace:
        try:
            res = run_bass_kernel_spmd(
                nc, in_maps, core_ids=core_ids, trace=True, trace_cores=[0]
            )
        except Exception as e:  # fall back to an untraced run
            print(f"[kernel] traced run failed ({type(e).__name__}: {e}); retrying untraced")
            res = run_bass_kernel_spmd(nc, in_maps, core_ids=core_ids)
    else:
        res = run_bass_kernel_spmd(nc, in_maps, core_ids=core_ids)
    LAST_RESULTS = res
    out = np.concatenate([r["out"] for r in res.results], axis=0)
    return out[: g.n_nodes]


_FULL = Geom(
    n_nodes=100000,
    n_cores=8,
    in_dim=128,
    h1=128,
    h2=64,
    gcols1=int(os.environ.get("GCN_GCOLS1", "7")),
    gcols2=int(os.environ.get("GCN_GCOLS2", "7")),
    selfk=int(os.environ.get("GCN_SELFK", "7")),
    mm_bf16=os.environ.get("GCN_F32", "0") != "1",
)


def kernel(x, edge_index, W1, b1, W2, b2):
    trace = os.environ.get("GCN_TRACE", "0") == "1"
    return run(x, edge_index, W1, b1, W2, b2, _FULL, trace=trace)
